# revision 2
# baseline (speedup 1.0000x reference)
"""MultiHeadAttention forward on 8 Trainium2 NeuronCores — v3.

Problem: x[2,2048,1024] -> fused QKV proj -> 16-head attention -> out proj.
Sharding: (batch=2) x (head-groups=4) across 8 cores; core c handles batch
c//4 and heads 4*(c%4)..4*(c%4)+3. Host sums the 4 head-group partials.

Structure (optimized against the TimelineSim cost model, bf16 precision —
measured: fp8 projections/P/V all pass their elementwise error ~1:1 into the
attention output, so only bf16 survives the 2e-2 gate):
  - QKV projection bf16, feature-major q/k (scores operands), token-major V.
  - scores bf16 -> fp32 PSUM in [128,512] half-tiles; two decoupled 3-slot
    PSUM pools so the two exp engines pipeline independently.
  - exp split by key-tile parity: even ks -> ACT true exp; odd ks -> DVE
    fast-exp (one tensor_scalar mult+add -> int16 bits == bf16 exp values).
  - PV with queries on PSUM partitions: lhsT = exp-tile, rhs = V augmented
    with a ones column -> softmax denominators land as output column 64;
    normalize = reciprocal [128,1] + per-partition scalar multiply.
  - attn [q,f] -> PE transpose (identity) -> attnT [f,q] -> out-proj bf16.
  - b_v and b_out are folded on the host (softmax rows sum to 1, so both
    commute through attention into a constant added post-gather).
"""

import os
import numpy as np
import ml_dtypes

import concourse.bass as bass
import concourse.bacc as bacc
import concourse.tile as tile
from concourse import mybir
from concourse.alu_op_type import AluOpType
from concourse.bass_utils import run_bass_kernel_spmd

BF16 = ml_dtypes.bfloat16

B, S, E = 2, 2048, 1024
H, D = 16, 64
HG = 4               # heads per core
N_CORES = 8
P = 128
ST = S // P          # 16 token-tiles
ET = E // P          # 8 e-tiles

F32 = mybir.dt.float32
BF = mybir.dt.bfloat16
I16 = mybir.dt.int16
EXP = mybir.ActivationFunctionType.Exp
IDENT = mybir.ActivationFunctionType.Identity
COPY = mybir.ActivationFunctionType.Copy

ESC = 0.125          # exp(s_psum * ESC + EBIAS)
EBIAS = -2.0
LOG2E = 1.4426950408889634
# DVE fast-exp: u16 = round(s*K1 + K2); int16 bits == bf16(~exp(s/8 - 2))
FEXP_K1 = ESC * LOG2E * 128.0
FEXP_K2 = (127.0 + EBIAS * LOG2E) * 128.0 - 5.7

ALLACT = bool(os.environ.get("ALLACT"))

_COMPILED = None


def build_program():
    nc = bacc.Bacc("TRN2", target_bir_lowering=False, debug=False)

    xT_d = nc.dram_tensor("xT", [P, ET, S], BF, kind="ExternalInput").ap()
    wqk_d = nc.dram_tensor("wqk", [P, ET, 512], BF, kind="ExternalInput").ap()
    wv_d = nc.dram_tensor("wv", [P, ET, 256], BF, kind="ExternalInput").ap()
    wout_d = nc.dram_tensor("wout", [P, 2, E], BF, kind="ExternalInput").ap()
    bqk_d = nc.dram_tensor("bqk", [P, 4], F32, kind="ExternalInput").ap()
    ident_d = nc.dram_tensor("ident", [P, P], BF, kind="ExternalInput").ap()
    out_d = nc.dram_tensor("out", [S, E], F32, kind="ExternalOutput").ap()

    with tile.TileContext(nc) as tc:
        with (
            tc.tile_pool(name="consts", bufs=1) as consts,
            tc.tile_pool(name="qkt", bufs=1) as qkt_pool,
            tc.tile_pool(name="exp", bufs=20) as ex_pool,
            tc.tile_pool(name="attnq", bufs=9) as attnq_pool,
            tc.tile_pool(name="attnt", bufs=1) as attnt_pool,
            tc.tile_pool(name="rbp", bufs=6) as rbp,
            tc.tile_pool(name="outsb", bufs=4) as outsb,
            tc.tile_pool(name="psS", bufs=3, space="PSUM") as psS,
            tc.tile_pool(name="psD", bufs=3, space="PSUM") as psD,
            tc.tile_pool(name="psV", bufs=2, space="PSUM") as psV,
        ):
            # ---------------- constants / inputs ----------------
            wqk = consts.tile([P, ET, 512], BF, tag="wqk")
            nc.sync.dma_start(wqk[:, 0:4], wqk_d[:, 0:4])
            bqk_sb = consts.tile([P, 4], F32, tag="bqk")
            nc.gpsimd.dma_start(bqk_sb, bqk_d)
            xT = consts.tile([P, ET, S], BF, tag="xT", name="xT")
            for e2 in range(4):
                (nc.sync if e2 % 2 else nc.gpsimd).dma_start(
                    xT[:, 2 * e2:2 * e2 + 2], xT_d[:, 2 * e2:2 * e2 + 2])
            nc.sync.dma_start(wqk[:, 4:8], wqk_d[:, 4:8])
            wv = consts.tile([P, ET, 256], BF, tag="wv")
            nc.gpsimd.dma_start(wv, wv_d)
            wout_all = consts.tile([P, 2, E], BF, tag="wout")
            nc.sync.dma_start(wout_all, wout_d)
            wout_sb = [wout_all[:, c, :] for c in range(2)]
            ident = consts.tile([P, P], BF, tag="ident")
            nc.gpsimd.dma_start(ident, ident_d)
            ebias = consts.tile([P, 1], F32, tag="ebias")
            nc.vector.memset(ebias, EBIAS)
            warm = consts.tile([P, 1], BF, tag="warm")
            nc.scalar.activation(warm, ebias, EXP)

            # persistent activations
            # qkT m-tiles: 0=q(h0,h1) 1=q(h2,h3) 2=k(h0,h1) 3=k(h2,h3);
            # partitions 0:64 = even head, 64:128 = odd head; bf16.
            qkT = [[qkt_pool.tile([P, 512], BF, tag=f"qkT{m}_{s4}",
                                  name=f"qkT{m}_{s4}")
                    for s4 in range(4)] for m in range(4)]
            # V augmented: [128 tok, st, head, 65]; col 64 = 1.0 (denom row)
            vaug = consts.tile([P, ST, HG, 65], BF, tag="vaug")
            nc.vector.memset(vaug[:, :, :, 64:65], 1.0)
            # attnT [f, q] for out-proj: c=0 heads {0,1}, c=1 heads {2,3}
            attnT = [attnt_pool.tile([P, S], BF, tag=f"attnT{c}",
                                     name=f"attnT{c}") for c in range(2)]

            # ---------------- emission pieces ----------------
            qk_rot = [0]

            def qk_proj(s4, m):
                rot = qk_rot[0]
                qk_rot[0] = (qk_rot[0] + 1) % ET
                ps = (psS if (s4 + m) % 2 else psD).tile(
                    [P, 512], F32, tag="sc", name=f"qk{s4}_{m}")
                for i in range(ET):
                    e = (rot + i) % ET
                    nc.tensor.matmul(
                        ps, lhsT=wqk[:, e, m * P:(m + 1) * P],
                        rhs=xT[:, e, s4 * 512:(s4 + 1) * 512],
                        start=(i == 0), stop=(i == ET - 1))
                nc.scalar.activation(qkT[m][s4], ps, IDENT,
                                     bias=bqk_sb[:, m:m + 1])

            def v_proj(st):
                psv = (psS if st % 2 else psD).tile(
                    [P, 512], F32, tag="sc", name=f"v{st}")
                for i in range(ET):
                    e = (qk_rot[0] + i) % ET
                    nc.tensor.matmul(
                        psv[:, 0:256], lhsT=xT[:, e, st * P:(st + 1) * P],
                        rhs=wv[:, e], start=(i == 0), stop=(i == ET - 1))
                nc.vector.tensor_copy(
                    vaug[:, st, :, 0:64],
                    psv[:, 0:256].rearrange("p (h d) -> p h d", h=HG))

            def scores_exp(h, q2, ks, ex):
                """scores for key-tile ks vs this q2's 1024 queries + exp.
                even ks -> ACT true exp; odd ks -> DVE fast-exp."""
                pair, hp = h // 2, h % 2
                qm, km = pair, 2 + pair
                bp = hp * 64
                pool = psS if ks % 2 == 0 else psD
                for qh in range(2):
                    scs = pool.tile([P, 512], F32, tag="sc",
                                    name=f"sc{h}{q2}{ks}{qh}")
                    nc.tensor.matmul(
                        scs,
                        lhsT=qkT[km][ks // 4][bp:bp + 64,
                                              (ks % 4) * P:(ks % 4 + 1) * P],
                        rhs=qkT[qm][q2 * 2 + qh][bp:bp + 64, :],
                        start=True, stop=True)
                    qsl = slice(qh * 512, (qh + 1) * 512)
                    if ks % 2 == 0 or ALLACT:
                        nc.scalar.activation(ex[:, qsl], scs, EXP,
                                             bias=ebias, scale=ESC)
                    else:
                        nc.vector.tensor_scalar(
                            ex.bitcast(I16)[:, qsl], scs,
                            FEXP_K1, FEXP_K2, AluOpType.mult, AluOpType.add)

            deferred = []

            def attn_head(h, q2, exs, filler=None):
                for kp in range(ST // 2):
                    for j in range(2):
                        ex = ex_pool.tile([P, 1024], BF, tag="ex", name="ex")
                        exs.append(ex)
                        scores_exp(h, q2, 2 * kp + j, ex)
                    if kp == 0 and deferred:
                        deferred.pop(0)()
                    if filler:
                        if kp == 7:
                            deferred.append(lambda f=filler: f(7))
                        else:
                            filler(kp)

            def pv_norm(h, q2, qt, exs):
                """PV + normalize for one query tile (128 q) of head h."""
                qs = slice((qt % 8) * P, (qt % 8 + 1) * P)
                pv = psV.tile([P, 512], F32, tag="pv", name=f"pv{h}{q2}{qt}")
                for ks, ex in enumerate(exs):
                    nc.tensor.matmul(
                        pv[:, 0:65], lhsT=ex[:, qs],
                        rhs=vaug[:, ks, h, :],
                        start=(ks == 0), stop=(ks == ST - 1))
                rb = rbp.tile([P, 1], F32, tag="rb", name="rb")
                nc.vector.reciprocal_approx_fast(rb, pv[:, 64:65])
                aq = attnq[qt % 8]
                nc.vector.tensor_scalar(
                    aq[:, h * 64:(h + 1) * 64], pv[:, 0:64],
                    rb, None, AluOpType.mult)

            def transpose_qt(qt):
                aq = attnq[qt % 8]
                tp = psV.tile([P, 512], BF, tag="pv", name=f"tp{qt}")
                for c in range(2):
                    nc.tensor.transpose(
                        tp[:, c * P:(c + 1) * P], aq[:, c * P:(c + 1) * P],
                        ident)
                for c in range(2):
                    nc.vector.tensor_copy(
                        attnT[c][:, qt * P:(qt + 1) * P],
                        tp[:, c * P:(c + 1) * P])

            def out_proj(qt, e2):
                po = (psS if e2 else psD).tile(
                    [P, 512], F32, tag="sc", name=f"po{qt}_{e2}")
                for i, c in enumerate((0, 1)):
                    nc.tensor.matmul(
                        po, lhsT=attnT[c][:, qt * P:(qt + 1) * P],
                        rhs=wout_sb[c][:, e2 * 512:(e2 + 1) * 512],
                        start=(i == 0), stop=(i == 1))
                ob = outsb.tile([P, 512], F32, tag="ob", name="ob")
                nc.scalar.activation(ob, po, COPY)
                (nc.sync if (qt + e2) % 2 else nc.gpsimd).dma_start(
                    out_d[qt * P:(qt + 1) * P, e2 * 512:(e2 + 1) * 512], ob)

            # ---------------- schedule ----------------
            attnq = [attnq_pool.tile([P, 256], BF, tag="aq", name=f"aq{i}")
                     for i in range(8)]

            for s4 in range(4):
                qk_proj(s4, 0)
            for s4 in range(4):
                qk_proj(s4, 2)

            ex_streams = {}
            done_pv = []
            tp_queue = []

            def drain_tp(n):
                for _ in range(n):
                    if tp_queue:
                        qt = tp_queue.pop(0)
                        transpose_qt(qt)
                        out_proj(qt, 0)
                        out_proj(qt, 1)

            for q2 in range(2):
                for h in range(4):
                    exs = []
                    ex_streams[(h, q2)] = exs

                    def filler(kp, h=h, q2=q2):
                        if q2 == 0 and h == 0:
                            if kp < 4:
                                qk_proj(kp, 1)
                                qk_proj(kp, 3)
                            else:
                                for st in range(4 * (kp - 4), 4 * (kp - 3)):
                                    v_proj(st)
                            return
                        drain_tp(1)
                        prev = (h - 1, q2) if h > 0 else (3, 0)
                        if prev in ex_streams and prev not in done_pv:
                            ph, pq2 = prev
                            qt = pq2 * 8 + kp
                            pv_norm(ph, pq2, qt, ex_streams[prev])
                            if ph == 3:
                                tp_queue.append(qt)
                            if kp == 7:
                                done_pv.append(prev)

                    attn_head(h, q2, exs, filler)

            # tail: per-qt chain of PV+norm -> transpose -> out-proj
            while deferred:
                deferred.pop(0)()
            drain_tp(len(tp_queue))
            for qt in range(8, 16):
                pv_norm(3, 1, qt, ex_streams[(3, 1)])
                transpose_qt(qt)
                out_proj(qt, 0)
                out_proj(qt, 1)

    nc.compile()
    return nc


def get_program():
    global _COMPILED
    if _COMPILED is None:
        _COMPILED = build_program()
    return _COMPILED


def make_in_maps(x, W_qkv, b_qkv, W_out, b_out):
    x = np.asarray(x, dtype=np.float32)
    W_qkv = np.asarray(W_qkv, dtype=np.float32)
    b_qkv = np.asarray(b_qkv, dtype=np.float32)
    W_out = np.asarray(W_out, dtype=np.float32)

    ident = np.eye(P, dtype=np.float32).astype(BF16)
    in_maps = []
    for c in range(N_CORES):
        b = c // 4
        g = c % 4
        heads = [4 * g + i for i in range(HG)]
        xT = np.ascontiguousarray(
            x[b].T.reshape(ET, P, S).transpose(1, 0, 2)).astype(BF16)

        qcols = np.empty((E, 512), np.float32)
        bqk = np.empty((P, 4), np.float32)
        vcols = np.empty((E, 256), np.float32)
        wout = np.empty((P, 2, E), np.float32)
        for m in range(4):
            for hl2 in range(2):
                head = heads[(m % 2) * 2 + hl2]
                base = head * 3 * D + (0 if m < 2 else D)
                qcols[:, m * P + hl2 * D: m * P + (hl2 + 1) * D] = \
                    W_qkv[:, base:base + D]
                bqk[hl2 * D:(hl2 + 1) * D, m] = b_qkv[base:base + D]
        for hl in range(HG):
            base = heads[hl] * 3 * D + 2 * D
            vcols[:, hl * D:(hl + 1) * D] = W_qkv[:, base:base + D]
        for cc in range(2):
            for f in range(P):
                wout[f, cc, :] = W_out[(heads[cc * 2 + f // D]) * D + f % D, :]

        wqk = np.ascontiguousarray(
            qcols.reshape(ET, P, 512).transpose(1, 0, 2)).astype(BF16)
        wv = np.ascontiguousarray(
            vcols.reshape(ET, P, 256).transpose(1, 0, 2)).astype(BF16)

        in_maps.append({
            "xT": xT,
            "wqk": wqk,
            "wv": wv,
            "wout": np.ascontiguousarray(wout).astype(BF16),
            "bqk": np.ascontiguousarray(bqk),
            "ident": ident,
        })
    return in_maps


def fold_bias(W_qkv, b_qkv, W_out, b_out):
    """b_out plus the V-bias contribution: softmax rows sum to 1, so each
    head's b_v passes straight through attention into the out-projection."""
    bv_all = b_qkv.reshape(H, 3 * D)[:, 2 * D:].reshape(E)
    return (b_out + bv_all @ W_out).astype(np.float32)


def gather_outputs(results, bias_const):
    out = np.zeros((B, S, E), np.float32)
    for c in range(N_CORES):
        out[c // 4] += results[c]["out"]
    return out + bias_const


def run(in_maps, trace=False, **kwargs):
    nc = get_program()
    return run_bass_kernel_spmd(nc, in_maps, list(range(N_CORES)),
                                trace=trace, **kwargs)


def kernel(x, W_qkv, b_qkv, W_out, b_out):
    W_qkv = np.asarray(W_qkv, np.float32)
    b_qkv = np.asarray(b_qkv, np.float32)
    W_out = np.asarray(W_out, np.float32)
    b_out = np.asarray(b_out, np.float32)
    in_maps = make_in_maps(x, W_qkv, b_qkv, W_out, b_out)
    res = run(in_maps)
    return gather_outputs(res.results, fold_bias(W_qkv, b_qkv, W_out, b_out))


# revision 3
# speedup vs baseline: 1.0743x; 1.0743x over previous
"""MultiHeadAttention forward on 8 Trainium2 NeuronCores — v3.

Problem: x[2,2048,1024] -> fused QKV proj -> 16-head attention -> out proj.
Sharding: (batch=2) x (head-groups=4) across 8 cores; core c handles batch
c//4 and heads 4*(c%4)..4*(c%4)+3. Host sums the 4 head-group partials.

Structure (optimized against the TimelineSim cost model, bf16 precision —
measured: fp8 projections/P/V all pass their elementwise error ~1:1 into the
attention output, so only bf16 survives the 2e-2 gate):
  - QKV projection bf16, feature-major q/k (scores operands), token-major V.
  - scores bf16 -> fp32 PSUM in [128,512] half-tiles; two decoupled 3-slot
    PSUM pools so the two exp engines pipeline independently.
  - exp split by key-tile parity: even ks -> ACT true exp; odd ks -> DVE
    fast-exp (one tensor_scalar mult+add -> int16 bits == bf16 exp values).
  - PV with queries on PSUM partitions: lhsT = exp-tile, rhs = V augmented
    with a ones column -> softmax denominators land as output column 64;
    normalize = reciprocal [128,1] + per-partition scalar multiply.
  - attn [q,f] -> PE transpose (identity) -> attnT [f,q] -> out-proj bf16.
  - b_v and b_out are folded on the host (softmax rows sum to 1, so both
    commute through attention into a constant added post-gather).
"""

import os
import numpy as np
import ml_dtypes

import concourse.bass as bass
import concourse.bacc as bacc
import concourse.tile as tile
from concourse import mybir
from concourse.alu_op_type import AluOpType
from concourse.bass_utils import run_bass_kernel_spmd

BF16 = ml_dtypes.bfloat16

B, S, E = 2, 2048, 1024
H, D = 16, 64
HG = 4               # heads per core
N_CORES = 8
P = 128
ST = S // P          # 16 token-tiles
ET = E // P          # 8 e-tiles

F32 = mybir.dt.float32
BF = mybir.dt.bfloat16
I16 = mybir.dt.int16
EXP = mybir.ActivationFunctionType.Exp
IDENT = mybir.ActivationFunctionType.Identity
COPY = mybir.ActivationFunctionType.Copy

ESC = 0.125          # exp(s_psum * ESC + EBIAS)
EBIAS = -2.0
LOG2E = 1.4426950408889634
# DVE fast-exp: u16 = round(s*K1 + K2); int16 bits == bf16(~exp(s/8 - 2))
FEXP_K1 = ESC * LOG2E * 128.0
FEXP_K2 = (127.0 + EBIAS * LOG2E) * 128.0 - 5.7

ALLACT = bool(os.environ.get("ALLACT"))

_COMPILED = None


def build_program():
    nc = bacc.Bacc("TRN2", target_bir_lowering=False, debug=False)

    xT_d = nc.dram_tensor("xT", [P, ET, S], BF, kind="ExternalInput").ap()
    wqk_d = nc.dram_tensor("wqk", [P, ET, 512], BF, kind="ExternalInput").ap()
    wv_d = nc.dram_tensor("wv", [P, ET, 256], BF, kind="ExternalInput").ap()
    wout_d = nc.dram_tensor("wout", [P, 2, E], BF, kind="ExternalInput").ap()
    bqk_d = nc.dram_tensor("bqk", [P, 4], F32, kind="ExternalInput").ap()
    ident_d = nc.dram_tensor("ident", [P, P], BF, kind="ExternalInput").ap()
    out_d = nc.dram_tensor("out", [S, E], F32, kind="ExternalOutput").ap()

    with tile.TileContext(nc) as tc:
        with (
            tc.tile_pool(name="consts", bufs=1) as consts,
            tc.tile_pool(name="qkt", bufs=1) as qkt_pool,
            tc.tile_pool(name="exp", bufs=34) as ex_pool,
            tc.tile_pool(name="attnq", bufs=9) as attnq_pool,
            tc.tile_pool(name="attnt", bufs=1) as attnt_pool,
            tc.tile_pool(name="rbp", bufs=6) as rbp,
            tc.tile_pool(name="outsb", bufs=4) as outsb,
            tc.tile_pool(name="psS", bufs=3, space="PSUM") as psS,
            tc.tile_pool(name="psD", bufs=3, space="PSUM") as psD,
            tc.tile_pool(name="psV", bufs=2, space="PSUM") as psV,
        ):
            # ---------------- constants / inputs ----------------
            wqk = consts.tile([P, ET, 512], BF, tag="wqk")
            nc.sync.dma_start(wqk[:, 0:4], wqk_d[:, 0:4])
            bqk_sb = consts.tile([P, 4], F32, tag="bqk")
            nc.gpsimd.dma_start(bqk_sb, bqk_d)
            xT = consts.tile([P, ET, S], BF, tag="xT", name="xT")
            for e2 in range(4):
                (nc.sync if e2 % 2 else nc.gpsimd).dma_start(
                    xT[:, 2 * e2:2 * e2 + 2], xT_d[:, 2 * e2:2 * e2 + 2])
            nc.sync.dma_start(wqk[:, 4:8], wqk_d[:, 4:8])
            wv = consts.tile([P, ET, 256], BF, tag="wv")
            nc.gpsimd.dma_start(wv, wv_d)
            wout_all = consts.tile([P, 2, E], BF, tag="wout")
            nc.sync.dma_start(wout_all, wout_d)
            wout_sb = [wout_all[:, c, :] for c in range(2)]
            ident = consts.tile([P, P], BF, tag="ident")
            nc.gpsimd.dma_start(ident, ident_d)
            ebias = consts.tile([P, 1], F32, tag="ebias")
            nc.vector.memset(ebias, EBIAS)
            warm = consts.tile([P, 1], BF, tag="warm")
            nc.scalar.activation(warm, ebias, EXP)

            # persistent activations
            # qkT m-tiles: 0=q(h0,h1) 1=q(h2,h3) 2=k(h0,h1) 3=k(h2,h3);
            # partitions 0:64 = even head, 64:128 = odd head; bf16.
            qkT = [[qkt_pool.tile([P, 512], BF, tag=f"qkT{m}_{s4}",
                                  name=f"qkT{m}_{s4}")
                    for s4 in range(4)] for m in range(4)]
            # V augmented: [128 tok, st, head, 65]; col 64 = 1.0 (denom row)
            vaug = consts.tile([P, ST, HG, 65], BF, tag="vaug")
            nc.vector.memset(vaug[:, :, :, 64:65], 1.0)
            # attnT [f, q] for out-proj: c=0 heads {0,1}, c=1 heads {2,3}
            attnT = [attnt_pool.tile([P, S], BF, tag=f"attnT{c}",
                                     name=f"attnT{c}") for c in range(2)]

            # ---------------- emission pieces ----------------
            qk_rot = [0]

            def qk_proj(s4, m):
                rot = qk_rot[0]
                qk_rot[0] = (qk_rot[0] + 1) % ET
                ps = (psS if (s4 + m) % 2 else psD).tile(
                    [P, 512], F32, tag="sc", name=f"qk{s4}_{m}")
                for i in range(ET):
                    e = (rot + i) % ET
                    nc.tensor.matmul(
                        ps, lhsT=wqk[:, e, m * P:(m + 1) * P],
                        rhs=xT[:, e, s4 * 512:(s4 + 1) * 512],
                        start=(i == 0), stop=(i == ET - 1))
                nc.scalar.activation(qkT[m][s4], ps, IDENT,
                                     bias=bqk_sb[:, m:m + 1])

            def v_proj(st):
                psv = (psS if st % 2 else psD).tile(
                    [P, 512], F32, tag="sc", name=f"v{st}")
                for i in range(ET):
                    e = (qk_rot[0] + i) % ET
                    nc.tensor.matmul(
                        psv[:, 0:256], lhsT=xT[:, e, st * P:(st + 1) * P],
                        rhs=wv[:, e], start=(i == 0), stop=(i == ET - 1))
                nc.vector.tensor_copy(
                    vaug[:, st, :, 0:64],
                    psv[:, 0:256].rearrange("p (h d) -> p h d", h=HG))

            def scores_exp(h, q2, ks, ex):
                """scores for key-tile ks vs this q2's 1024 queries + exp.
                even ks -> ACT true exp; odd ks -> DVE fast-exp."""
                pair, hp = h // 2, h % 2
                qm, km = pair, 2 + pair
                bp = hp * 64
                pool = psS if ks % 2 == 0 else psD
                for qh in range(2):
                    scs = pool.tile([P, 512], F32, tag="sc",
                                    name=f"sc{h}{q2}{ks}{qh}")
                    nc.tensor.matmul(
                        scs,
                        lhsT=qkT[km][ks // 4][bp:bp + 64,
                                              (ks % 4) * P:(ks % 4 + 1) * P],
                        rhs=qkT[qm][q2 * 2 + qh][bp:bp + 64, :],
                        start=True, stop=True)
                    qsl = slice(qh * 512, (qh + 1) * 512)
                    if ks % 2 == 0 or ALLACT:
                        nc.scalar.activation(ex[:, qsl], scs, EXP,
                                             bias=ebias, scale=ESC)
                    else:
                        nc.vector.tensor_scalar(
                            ex.bitcast(I16)[:, qsl], scs,
                            FEXP_K1, FEXP_K2, AluOpType.mult, AluOpType.add)

            deferred = []

            def attn_head(h, q2, exs, filler=None):
                for kp in range(ST // 2):
                    for j in range(2):
                        ex = ex_pool.tile([P, 1024], BF, tag="ex", name="ex")
                        exs.append(ex)
                        scores_exp(h, q2, 2 * kp + j, ex)
                    if kp == 0 and deferred:
                        deferred.pop(0)()
                    if filler:
                        if kp == 7:
                            deferred.append(lambda f=filler: f(7))
                        else:
                            filler(kp)

            def pv_norm(h, q2, qt, exs):
                """PV + normalize for one query tile (128 q) of head h."""
                qs = slice((qt % 8) * P, (qt % 8 + 1) * P)
                pv = psV.tile([P, 512], F32, tag="pv", name=f"pv{h}{q2}{qt}")
                for ks, ex in enumerate(exs):
                    nc.tensor.matmul(
                        pv[:, 0:65], lhsT=ex[:, qs],
                        rhs=vaug[:, ks, h, :],
                        start=(ks == 0), stop=(ks == ST - 1))
                rb = rbp.tile([P, 1], F32, tag="rb", name="rb")
                nc.vector.reciprocal_approx_fast(rb, pv[:, 64:65])
                aq = attnq[qt % 8]
                nc.vector.tensor_scalar(
                    aq[:, h * 64:(h + 1) * 64], pv[:, 0:64],
                    rb, None, AluOpType.mult)

            def transpose_qt(qt):
                aq = attnq[qt % 8]
                tp = psD.tile([P, 512], BF, tag="sc", name=f"tp{qt}")
                for c in range(2):
                    nc.tensor.transpose(
                        tp[:, c * P:(c + 1) * P], aq[:, c * P:(c + 1) * P],
                        ident)
                for c in range(2):
                    nc.vector.tensor_copy(
                        attnT[c][:, qt * P:(qt + 1) * P],
                        tp[:, c * P:(c + 1) * P])

            def out_proj(qt, e2):
                po = (psS if e2 else psD).tile(
                    [P, 512], F32, tag="sc", name=f"po{qt}_{e2}")
                for i, c in enumerate((0, 1)):
                    nc.tensor.matmul(
                        po, lhsT=attnT[c][:, qt * P:(qt + 1) * P],
                        rhs=wout_sb[c][:, e2 * 512:(e2 + 1) * 512],
                        start=(i == 0), stop=(i == 1))
                ob = outsb.tile([P, 512], F32, tag="ob", name="ob")
                nc.scalar.activation(ob, po, COPY)
                (nc.sync if (qt + e2) % 2 else nc.gpsimd).dma_start(
                    out_d[qt * P:(qt + 1) * P, e2 * 512:(e2 + 1) * 512], ob)

            # ---------------- schedule ----------------
            attnq = [attnq_pool.tile([P, 256], BF, tag="aq", name=f"aq{i}")
                     for i in range(8)]

            for s4 in range(4):
                qk_proj(s4, 0)
            for s4 in range(4):
                qk_proj(s4, 2)

            ex_streams = {}
            done_pv = []
            tp_queue = []

            def drain_tp(n):
                for _ in range(n):
                    if tp_queue:
                        qt = tp_queue.pop(0)
                        transpose_qt(qt)
                        out_proj(qt, 0)
                        out_proj(qt, 1)

            for q2 in range(2):
                for h in range(4):
                    exs = []
                    ex_streams[(h, q2)] = exs

                    def filler(kp, h=h, q2=q2):
                        if q2 == 0 and h == 0:
                            if kp < 4:
                                qk_proj(kp, 1)
                                qk_proj(kp, 3)
                            else:
                                for st in range(4 * (kp - 4), 4 * (kp - 3)):
                                    v_proj(st)
                            return
                        drain_tp(1)
                        prev = (h - 1, q2) if h > 0 else (3, 0)
                        if prev in ex_streams and prev not in done_pv:
                            ph, pq2 = prev
                            qt = pq2 * 8 + kp
                            pv_norm(ph, pq2, qt, ex_streams[prev])
                            if ph == 3:
                                tp_queue.append(qt)
                            if kp == 7:
                                done_pv.append(prev)

                    attn_head(h, q2, exs, filler)

            # tail: per-qt chain of PV+norm -> transpose -> out-proj
            while deferred:
                deferred.pop(0)()
            drain_tp(len(tp_queue))
            for qt in range(8, 16):
                pv_norm(3, 1, qt, ex_streams[(3, 1)])
                transpose_qt(qt)
                out_proj(qt, 0)
                out_proj(qt, 1)

    nc.compile()
    return nc


def get_program():
    global _COMPILED
    if _COMPILED is None:
        _COMPILED = build_program()
    return _COMPILED


def make_in_maps(x, W_qkv, b_qkv, W_out, b_out):
    x = np.asarray(x, dtype=np.float32)
    W_qkv = np.asarray(W_qkv, dtype=np.float32)
    b_qkv = np.asarray(b_qkv, dtype=np.float32)
    W_out = np.asarray(W_out, dtype=np.float32)

    ident = np.eye(P, dtype=np.float32).astype(BF16)
    in_maps = []
    for c in range(N_CORES):
        b = c // 4
        g = c % 4
        heads = [4 * g + i for i in range(HG)]
        xT = np.ascontiguousarray(
            x[b].T.reshape(ET, P, S).transpose(1, 0, 2)).astype(BF16)

        qcols = np.empty((E, 512), np.float32)
        bqk = np.empty((P, 4), np.float32)
        vcols = np.empty((E, 256), np.float32)
        wout = np.empty((P, 2, E), np.float32)
        for m in range(4):
            for hl2 in range(2):
                head = heads[(m % 2) * 2 + hl2]
                base = head * 3 * D + (0 if m < 2 else D)
                qcols[:, m * P + hl2 * D: m * P + (hl2 + 1) * D] = \
                    W_qkv[:, base:base + D]
                bqk[hl2 * D:(hl2 + 1) * D, m] = b_qkv[base:base + D]
        for hl in range(HG):
            base = heads[hl] * 3 * D + 2 * D
            vcols[:, hl * D:(hl + 1) * D] = W_qkv[:, base:base + D]
        for cc in range(2):
            for f in range(P):
                wout[f, cc, :] = W_out[(heads[cc * 2 + f // D]) * D + f % D, :]

        wqk = np.ascontiguousarray(
            qcols.reshape(ET, P, 512).transpose(1, 0, 2)).astype(BF16)
        wv = np.ascontiguousarray(
            vcols.reshape(ET, P, 256).transpose(1, 0, 2)).astype(BF16)

        in_maps.append({
            "xT": xT,
            "wqk": wqk,
            "wv": wv,
            "wout": np.ascontiguousarray(wout).astype(BF16),
            "bqk": np.ascontiguousarray(bqk),
            "ident": ident,
        })
    return in_maps


def fold_bias(W_qkv, b_qkv, W_out, b_out):
    """b_out plus the V-bias contribution: softmax rows sum to 1, so each
    head's b_v passes straight through attention into the out-projection."""
    bv_all = b_qkv.reshape(H, 3 * D)[:, 2 * D:].reshape(E)
    return (b_out + bv_all @ W_out).astype(np.float32)


def gather_outputs(results, bias_const):
    out = np.zeros((B, S, E), np.float32)
    for c in range(N_CORES):
        out[c // 4] += results[c]["out"]
    return out + bias_const


def run(in_maps, trace=False, **kwargs):
    nc = get_program()
    return run_bass_kernel_spmd(nc, in_maps, list(range(N_CORES)),
                                trace=trace, **kwargs)


def kernel(x, W_qkv, b_qkv, W_out, b_out):
    W_qkv = np.asarray(W_qkv, np.float32)
    b_qkv = np.asarray(b_qkv, np.float32)
    W_out = np.asarray(W_out, np.float32)
    b_out = np.asarray(b_out, np.float32)
    in_maps = make_in_maps(x, W_qkv, b_qkv, W_out, b_out)
    res = run(in_maps)
    return gather_outputs(res.results, fold_bias(W_qkv, b_qkv, W_out, b_out))


# revision 4
# speedup vs baseline: 1.1206x; 1.0431x over previous
"""MultiHeadAttention forward on 8 Trainium2 NeuronCores — v3.

Problem: x[2,2048,1024] -> fused QKV proj -> 16-head attention -> out proj.
Sharding: (batch=2) x (head-groups=4) across 8 cores; core c handles batch
c//4 and heads 4*(c%4)..4*(c%4)+3. Host sums the 4 head-group partials.

Structure (optimized against the TimelineSim cost model, bf16 precision —
measured: fp8 projections/P/V all pass their elementwise error ~1:1 into the
attention output, so only bf16 survives the 2e-2 gate):
  - QKV projection bf16, feature-major q/k (scores operands), token-major V.
  - scores bf16 -> fp32 PSUM in [128,512] half-tiles; two decoupled 3-slot
    PSUM pools so the two exp engines pipeline independently.
  - exp split by key-tile parity: even ks -> ACT true exp; odd ks -> DVE
    fast-exp (one tensor_scalar mult+add -> int16 bits == bf16 exp values).
  - PV with queries on PSUM partitions: lhsT = exp-tile, rhs = V augmented
    with a ones column -> softmax denominators land as output column 64;
    normalize = reciprocal [128,1] + per-partition scalar multiply.
  - attn [q,f] -> PE transpose (identity) -> attnT [f,q] -> out-proj bf16.
  - b_v and b_out are folded on the host (softmax rows sum to 1, so both
    commute through attention into a constant added post-gather).
"""

import os
import numpy as np
import ml_dtypes

import concourse.bass as bass
import concourse.bacc as bacc
import concourse.tile as tile
from concourse import mybir
from concourse.alu_op_type import AluOpType
from concourse.bass_utils import run_bass_kernel_spmd

BF16 = ml_dtypes.bfloat16

B, S, E = 2, 2048, 1024
H, D = 16, 64
HG = 4               # heads per core
N_CORES = 8
P = 128
ST = S // P          # 16 token-tiles
ET = E // P          # 8 e-tiles

F32 = mybir.dt.float32
BF = mybir.dt.bfloat16
I16 = mybir.dt.int16
EXP = mybir.ActivationFunctionType.Exp
IDENT = mybir.ActivationFunctionType.Identity
COPY = mybir.ActivationFunctionType.Copy

ESC = 0.125          # exp(s_psum * ESC + EBIAS)
EBIAS = -2.0
LOG2E = 1.4426950408889634
# DVE fast-exp: u16 = round(s*K1 + K2); int16 bits == bf16(~exp(s/8 - 2))
FEXP_K1 = ESC * LOG2E * 128.0
FEXP_K2 = (127.0 + EBIAS * LOG2E) * 128.0 - 5.7

ALLACT = bool(os.environ.get("ALLACT"))

_COMPILED = None


def build_program():
    nc = bacc.Bacc("TRN2", target_bir_lowering=False, debug=False)

    xT_d = nc.dram_tensor("xT", [P, ET, S], BF, kind="ExternalInput").ap()
    wqk_d = nc.dram_tensor("wqk", [P, ET, 512], BF, kind="ExternalInput").ap()
    wv_d = nc.dram_tensor("wv", [P, ET, 256], BF, kind="ExternalInput").ap()
    wout_d = nc.dram_tensor("wout", [P, 2, E], BF, kind="ExternalInput").ap()
    bqk_d = nc.dram_tensor("bqk", [P, 4], F32, kind="ExternalInput").ap()
    ident_d = nc.dram_tensor("ident", [P, P], BF, kind="ExternalInput").ap()
    out_d = nc.dram_tensor("out", [S, E], F32, kind="ExternalOutput").ap()

    with tile.TileContext(nc) as tc:
        with (
            tc.tile_pool(name="consts", bufs=1) as consts,
            tc.tile_pool(name="qkt", bufs=1) as qkt_pool,
            tc.tile_pool(name="exp", bufs=34) as ex_pool,
            tc.tile_pool(name="attnq", bufs=9) as attnq_pool,
            tc.tile_pool(name="attnt", bufs=1) as attnt_pool,
            tc.tile_pool(name="rbp", bufs=14) as rbp,
            tc.tile_pool(name="outsb", bufs=12) as outsb,
            tc.tile_pool(name="psS", bufs=3, space="PSUM") as psS,
            tc.tile_pool(name="psD", bufs=3, space="PSUM") as psD,
            tc.tile_pool(name="psV", bufs=2, space="PSUM") as psV,
        ):
            # ---------------- constants / inputs ----------------
            wqk = consts.tile([P, ET, 512], BF, tag="wqk")
            nc.sync.dma_start(wqk[:, 0:4], wqk_d[:, 0:4])
            bqk_sb = consts.tile([P, 4], F32, tag="bqk")
            nc.gpsimd.dma_start(bqk_sb, bqk_d)
            xT = consts.tile([P, ET, S], BF, tag="xT", name="xT")
            for e2 in range(4):
                (nc.sync if e2 % 2 else nc.gpsimd).dma_start(
                    xT[:, 2 * e2:2 * e2 + 2], xT_d[:, 2 * e2:2 * e2 + 2])
            nc.sync.dma_start(wqk[:, 4:8], wqk_d[:, 4:8])
            wv = consts.tile([P, ET, 256], BF, tag="wv")
            nc.gpsimd.dma_start(wv, wv_d)
            wout_all = consts.tile([P, 2, E], BF, tag="wout")
            nc.sync.dma_start(wout_all, wout_d)
            wout_sb = [wout_all[:, c, :] for c in range(2)]
            ident = consts.tile([P, P], BF, tag="ident")
            nc.gpsimd.dma_start(ident, ident_d)
            ebias = consts.tile([P, 1], F32, tag="ebias")
            nc.vector.memset(ebias, EBIAS)
            warm = consts.tile([P, 1], BF, tag="warm")
            nc.scalar.activation(warm, ebias, EXP)

            # persistent activations
            # qkT m-tiles: 0=q(h0,h1) 1=q(h2,h3) 2=k(h0,h1) 3=k(h2,h3);
            # partitions 0:64 = even head, 64:128 = odd head; bf16.
            qkT = [[qkt_pool.tile([P, 512], BF, tag=f"qkT{m}_{s4}",
                                  name=f"qkT{m}_{s4}")
                    for s4 in range(4)] for m in range(4)]
            # V augmented: [128 tok, st, head, 65]; col 64 = 1.0 (denom row)
            vaug = consts.tile([P, ST, HG, 65], BF, tag="vaug")
            nc.vector.memset(vaug[:, :, :, 64:65], 1.0)
            # attnT [f, q] for out-proj: c=0 heads {0,1}, c=1 heads {2,3}
            attnT = [attnt_pool.tile([P, S], BF, tag=f"attnT{c}",
                                     name=f"attnT{c}") for c in range(2)]

            # ---------------- emission pieces ----------------
            qk_rot = [0]

            def qk_proj(s4, m):
                rot = qk_rot[0]
                qk_rot[0] = (qk_rot[0] + 1) % ET
                ps = (psS if (s4 + m) % 2 else psD).tile(
                    [P, 512], F32, tag="sc", name=f"qk{s4}_{m}")
                for i in range(ET):
                    e = (rot + i) % ET
                    nc.tensor.matmul(
                        ps, lhsT=wqk[:, e, m * P:(m + 1) * P],
                        rhs=xT[:, e, s4 * 512:(s4 + 1) * 512],
                        start=(i == 0), stop=(i == ET - 1))
                nc.scalar.activation(qkT[m][s4], ps, IDENT,
                                     bias=bqk_sb[:, m:m + 1])

            def v_proj(st):
                psv = (psS if st % 2 else psD).tile(
                    [P, 512], F32, tag="sc", name=f"v{st}")
                for i in range(ET):
                    e = (qk_rot[0] + i) % ET
                    nc.tensor.matmul(
                        psv[:, 0:256], lhsT=xT[:, e, st * P:(st + 1) * P],
                        rhs=wv[:, e], start=(i == 0), stop=(i == ET - 1))
                nc.vector.tensor_copy(
                    vaug[:, st, :, 0:64],
                    psv[:, 0:256].rearrange("p (h d) -> p h d", h=HG))

            def scores_exp_half(h, q2, ks, ex, qh):
                """one [128,512] score half + its exp.
                even ks -> ACT true exp; odd ks -> DVE fast-exp."""
                pair, hp = h // 2, h % 2
                qm, km = pair, 2 + pair
                bp = hp * 64
                pool = psS if ks % 2 == 0 else psD
                scs = pool.tile([P, 512], F32, tag="sc",
                                name=f"sc{h}{q2}{ks}{qh}")
                nc.tensor.matmul(
                    scs,
                    lhsT=qkT[km][ks // 4][bp:bp + 64,
                                          (ks % 4) * P:(ks % 4 + 1) * P],
                    rhs=qkT[qm][q2 * 2 + qh][bp:bp + 64, :],
                    start=True, stop=True)
                qsl = slice(qh * 512, (qh + 1) * 512)
                if ks % 2 == 0 or ALLACT:
                    nc.scalar.activation(ex[:, qsl], scs, EXP,
                                         bias=ebias, scale=ESC)
                else:
                    nc.vector.tensor_scalar(
                        ex.bitcast(I16)[:, qsl], scs,
                        FEXP_K1, FEXP_K2, AluOpType.mult, AluOpType.add)

            deferred = []

            def attn_head(h, q2, exs, filler=None):
                for kp in range(ST // 2):
                    exa = ex_pool.tile([P, 1024], BF, tag="ex", name="exa")
                    exd = ex_pool.tile([P, 1024], BF, tag="ex", name="exd")
                    exs.extend([exa, exd])
                    for qh in range(2):
                        scores_exp_half(h, q2, 2 * kp, exa, qh)
                        scores_exp_half(h, q2, 2 * kp + 1, exd, qh)
                    if kp == 0 and deferred:
                        deferred.pop(0)()
                    if filler:
                        if kp == 7:
                            deferred.append(lambda f=filler: f(7))
                        else:
                            filler(kp)

            def pv_norm(h, q2, qt, exs):
                """PV + normalize for one query tile (128 q) of head h."""
                qs = slice((qt % 8) * P, (qt % 8 + 1) * P)
                pv = psV.tile([P, 512], F32, tag="pv", name=f"pv{h}{q2}{qt}")
                for ks, ex in enumerate(exs):
                    nc.tensor.matmul(
                        pv[:, 0:65], lhsT=ex[:, qs],
                        rhs=vaug[:, ks, h, :],
                        start=(ks == 0), stop=(ks == ST - 1))
                rb = rbp.tile([P, 1], F32, tag="rb", name="rb")
                nc.vector.reciprocal_approx_fast(rb, pv[:, 64:65])
                aq = attnq[qt % 8]
                nc.vector.tensor_scalar(
                    aq[:, h * 64:(h + 1) * 64], pv[:, 0:64],
                    rb, None, AluOpType.mult)

            def transpose_qt(qt):
                aq = attnq[qt % 8]
                tp = psD.tile([P, 512], BF, tag="sc", name=f"tp{qt}")
                for c in range(2):
                    nc.tensor.transpose(
                        tp[:, c * P:(c + 1) * P], aq[:, c * P:(c + 1) * P],
                        ident)
                for c in range(2):
                    nc.vector.tensor_copy(
                        attnT[c][:, qt * P:(qt + 1) * P],
                        tp[:, c * P:(c + 1) * P])

            def out_proj(qt, e2):
                po = (psS if e2 else psD).tile(
                    [P, 512], F32, tag="sc", name=f"po{qt}_{e2}")
                for i, c in enumerate((0, 1)):
                    nc.tensor.matmul(
                        po, lhsT=attnT[c][:, qt * P:(qt + 1) * P],
                        rhs=wout_sb[c][:, e2 * 512:(e2 + 1) * 512],
                        start=(i == 0), stop=(i == 1))
                ob = outsb.tile([P, 512], F32, tag="ob", name="ob")
                nc.scalar.activation(ob, po, COPY)
                (nc.sync if (qt + e2) % 2 else nc.gpsimd).dma_start(
                    out_d[qt * P:(qt + 1) * P, e2 * 512:(e2 + 1) * 512], ob)

            # ---------------- schedule ----------------
            attnq = [attnq_pool.tile([P, 256], BF, tag="aq", name=f"aq{i}")
                     for i in range(8)]

            for s4 in range(4):
                qk_proj(s4, 0)
            for s4 in range(4):
                qk_proj(s4, 2)

            ex_streams = {}
            done_pv = []
            tp_queue = []

            def drain_tp(n):
                # fine-grained: one piece (transpose or one out-proj half)
                # per call so PE filler load stays smooth across kp slots
                for _ in range(n):
                    if tp_queue:
                        tp_queue.pop(0)()

            for q2 in range(2):
                for h in range(4):
                    exs = []
                    ex_streams[(h, q2)] = exs

                    def filler(kp, h=h, q2=q2):
                        if q2 == 0 and h == 0:
                            if kp < 4:
                                qk_proj(kp, 1)
                                qk_proj(kp, 3)
                            else:
                                for st in range(4 * (kp - 4), 4 * (kp - 3)):
                                    v_proj(st)
                            return
                        drain_tp(2)
                        prev = (h - 1, q2) if h > 0 else (3, 0)
                        if prev in ex_streams and prev not in done_pv:
                            ph, pq2 = prev
                            qt = pq2 * 8 + kp
                            pv_norm(ph, pq2, qt, ex_streams[prev])
                            if ph == 3:
                                tp_queue.extend([
                                    lambda q=qt: transpose_qt(q),
                                    lambda q=qt: out_proj(q, 0),
                                    lambda q=qt: out_proj(q, 1)])
                            if kp == 7:
                                done_pv.append(prev)

                    attn_head(h, q2, exs, filler)

            # tail: per-qt chain of PV+norm -> transpose -> out-proj
            while deferred:
                deferred.pop(0)()
            drain_tp(len(tp_queue))
            for qt in range(8, 16):
                pv_norm(3, 1, qt, ex_streams[(3, 1)])
                transpose_qt(qt)
                out_proj(qt, 0)
                out_proj(qt, 1)

    nc.compile()
    return nc


def get_program():
    global _COMPILED
    if _COMPILED is None:
        _COMPILED = build_program()
    return _COMPILED


def make_in_maps(x, W_qkv, b_qkv, W_out, b_out):
    x = np.asarray(x, dtype=np.float32)
    W_qkv = np.asarray(W_qkv, dtype=np.float32)
    b_qkv = np.asarray(b_qkv, dtype=np.float32)
    W_out = np.asarray(W_out, dtype=np.float32)

    ident = np.eye(P, dtype=np.float32).astype(BF16)
    in_maps = []
    for c in range(N_CORES):
        b = c // 4
        g = c % 4
        heads = [4 * g + i for i in range(HG)]
        xT = np.ascontiguousarray(
            x[b].T.reshape(ET, P, S).transpose(1, 0, 2)).astype(BF16)

        qcols = np.empty((E, 512), np.float32)
        bqk = np.empty((P, 4), np.float32)
        vcols = np.empty((E, 256), np.float32)
        wout = np.empty((P, 2, E), np.float32)
        for m in range(4):
            for hl2 in range(2):
                head = heads[(m % 2) * 2 + hl2]
                base = head * 3 * D + (0 if m < 2 else D)
                qcols[:, m * P + hl2 * D: m * P + (hl2 + 1) * D] = \
                    W_qkv[:, base:base + D]
                bqk[hl2 * D:(hl2 + 1) * D, m] = b_qkv[base:base + D]
        for hl in range(HG):
            base = heads[hl] * 3 * D + 2 * D
            vcols[:, hl * D:(hl + 1) * D] = W_qkv[:, base:base + D]
        for cc in range(2):
            for f in range(P):
                wout[f, cc, :] = W_out[(heads[cc * 2 + f // D]) * D + f % D, :]

        wqk = np.ascontiguousarray(
            qcols.reshape(ET, P, 512).transpose(1, 0, 2)).astype(BF16)
        wv = np.ascontiguousarray(
            vcols.reshape(ET, P, 256).transpose(1, 0, 2)).astype(BF16)

        in_maps.append({
            "xT": xT,
            "wqk": wqk,
            "wv": wv,
            "wout": np.ascontiguousarray(wout).astype(BF16),
            "bqk": np.ascontiguousarray(bqk),
            "ident": ident,
        })
    return in_maps


def fold_bias(W_qkv, b_qkv, W_out, b_out):
    """b_out plus the V-bias contribution: softmax rows sum to 1, so each
    head's b_v passes straight through attention into the out-projection."""
    bv_all = b_qkv.reshape(H, 3 * D)[:, 2 * D:].reshape(E)
    return (b_out + bv_all @ W_out).astype(np.float32)


def gather_outputs(results, bias_const):
    out = np.zeros((B, S, E), np.float32)
    for c in range(N_CORES):
        out[c // 4] += results[c]["out"]
    return out + bias_const


def run(in_maps, trace=False, **kwargs):
    nc = get_program()
    return run_bass_kernel_spmd(nc, in_maps, list(range(N_CORES)),
                                trace=trace, **kwargs)


def kernel(x, W_qkv, b_qkv, W_out, b_out):
    W_qkv = np.asarray(W_qkv, np.float32)
    b_qkv = np.asarray(b_qkv, np.float32)
    W_out = np.asarray(W_out, np.float32)
    b_out = np.asarray(b_out, np.float32)
    in_maps = make_in_maps(x, W_qkv, b_qkv, W_out, b_out)
    res = run(in_maps)
    return gather_outputs(res.results, fold_bias(W_qkv, b_qkv, W_out, b_out))


# revision 7
# speedup vs baseline: 1.1537x; 1.0295x over previous
"""MultiHeadAttention forward on 8 Trainium2 NeuronCores — v3.

Problem: x[2,2048,1024] -> fused QKV proj -> 16-head attention -> out proj.
Sharding: (batch=2) x (head-groups=4) across 8 cores; core c handles batch
c//4 and heads 4*(c%4)..4*(c%4)+3. Host sums the 4 head-group partials.

Structure (optimized against the TimelineSim cost model, bf16 precision —
measured: fp8 projections/P/V all pass their elementwise error ~1:1 into the
attention output, so only bf16 survives the 2e-2 gate):
  - QKV projection bf16, feature-major q/k (scores operands), token-major V.
  - scores bf16 -> fp32 PSUM in [128,512] half-tiles; two decoupled 3-slot
    PSUM pools so the two exp engines pipeline independently.
  - exp split by key-tile parity: even ks -> ACT true exp; odd ks -> DVE
    fast-exp (one tensor_scalar mult+add -> int16 bits == bf16 exp values).
  - PV with queries on PSUM partitions: lhsT = exp-tile, rhs = V augmented
    with a ones column -> softmax denominators land as output column 64;
    normalize = reciprocal [128,1] + per-partition scalar multiply.
  - attn [q,f] -> PE transpose (identity) -> attnT [f,q] -> out-proj bf16.
  - b_v and b_out are folded on the host (softmax rows sum to 1, so both
    commute through attention into a constant added post-gather).
"""

import os
import numpy as np
import ml_dtypes

import concourse.bass as bass
import concourse.bacc as bacc
import concourse.tile as tile
from concourse import mybir
from concourse.alu_op_type import AluOpType
from concourse.bass_utils import run_bass_kernel_spmd

BF16 = ml_dtypes.bfloat16

B, S, E = 2, 2048, 1024
H, D = 16, 64
HG = 4               # heads per core
N_CORES = 8
P = 128
ST = S // P          # 16 token-tiles
ET = E // P          # 8 e-tiles

F32 = mybir.dt.float32
BF = mybir.dt.bfloat16
I16 = mybir.dt.int16
EXP = mybir.ActivationFunctionType.Exp
IDENT = mybir.ActivationFunctionType.Identity
COPY = mybir.ActivationFunctionType.Copy

ESC = 0.125          # exp(s_psum * ESC + EBIAS)
EBIAS = -2.0
LOG2E = 1.4426950408889634
# DVE fast-exp: u16 = round(s*K1 + K2); int16 bits == bf16(~exp(s/8 - 2))
FEXP_K1 = ESC * LOG2E * 128.0
FEXP_K2 = (127.0 + EBIAS * LOG2E) * 128.0 - 5.7

ALLACT = bool(os.environ.get("ALLACT"))

_COMPILED = None


def build_program():
    nc = bacc.Bacc("TRN2", target_bir_lowering=False, debug=False)

    xT_d = nc.dram_tensor("xT", [P, ET, S], BF, kind="ExternalInput").ap()
    wqk_d = nc.dram_tensor("wqk", [P, ET, 512], BF, kind="ExternalInput").ap()
    wv_d = nc.dram_tensor("wv", [P, ET, 256], BF, kind="ExternalInput").ap()
    wout_d = nc.dram_tensor("wout", [P, 2, E], BF, kind="ExternalInput").ap()
    bqk_d = nc.dram_tensor("bqk", [P, 4], F32, kind="ExternalInput").ap()
    ident_d = nc.dram_tensor("ident", [P, P], BF, kind="ExternalInput").ap()
    out_d = nc.dram_tensor("out", [S, E], F32, kind="ExternalOutput").ap()

    with tile.TileContext(nc) as tc:
        with (
            tc.tile_pool(name="consts", bufs=1) as consts,
            tc.tile_pool(name="qkt", bufs=1) as qkt_pool,
            tc.tile_pool(name="exp", bufs=34) as ex_pool,
            tc.tile_pool(name="attnq", bufs=9) as attnq_pool,
            tc.tile_pool(name="attnt", bufs=1) as attnt_pool,
            tc.tile_pool(name="rbp", bufs=14) as rbp,
            tc.tile_pool(name="outsb", bufs=12) as outsb,
            tc.tile_pool(name="psS", bufs=3, space="PSUM") as psS,
            tc.tile_pool(name="psD", bufs=3, space="PSUM") as psD,
            tc.tile_pool(name="psV", bufs=2, space="PSUM") as psV,
        ):
            # ---------------- constants / inputs ----------------
            wqk = consts.tile([P, ET, 512], BF, tag="wqk")
            nc.sync.dma_start(wqk[:, 0:4], wqk_d[:, 0:4])
            bqk_sb = consts.tile([P, 4], F32, tag="bqk")
            nc.gpsimd.dma_start(bqk_sb, bqk_d)
            xT = consts.tile([P, ET, S], BF, tag="xT", name="xT")
            for e2 in range(4):
                (nc.sync if e2 % 2 else nc.gpsimd).dma_start(
                    xT[:, 2 * e2:2 * e2 + 2], xT_d[:, 2 * e2:2 * e2 + 2])
            nc.sync.dma_start(wqk[:, 4:8], wqk_d[:, 4:8])
            wv = consts.tile([P, ET, 256], BF, tag="wv")
            nc.gpsimd.dma_start(wv, wv_d)
            wout_all = consts.tile([P, 2, E], BF, tag="wout")
            nc.sync.dma_start(wout_all, wout_d)
            wout_sb = [wout_all[:, c, :] for c in range(2)]
            ident = consts.tile([P, P], BF, tag="ident")
            nc.gpsimd.dma_start(ident, ident_d)
            ebias = consts.tile([P, 1], F32, tag="ebias")
            nc.vector.memset(ebias, EBIAS)
            warm = consts.tile([P, 1], BF, tag="warm")
            nc.scalar.activation(warm, ebias, EXP)

            # persistent activations
            # qkT m-tiles: 0=q(h0,h1) 1=q(h2,h3) 2=k(h0,h1) 3=k(h2,h3);
            # partitions 0:64 = even head, 64:128 = odd head; bf16.
            qkT = [[qkt_pool.tile([P, 512], BF, tag=f"qkT{m}_{s4}",
                                  name=f"qkT{m}_{s4}")
                    for s4 in range(4)] for m in range(4)]
            # V augmented: [128 tok, st, head, 65]; col 64 = 1.0 (denom row)
            vaug = consts.tile([P, ST, HG, 65], BF, tag="vaug")
            nc.vector.memset(vaug[:, :, :, 64:65], 1.0)
            # attnT [f, q] for out-proj: c=0 heads {0,1}, c=1 heads {2,3}
            attnT = [attnt_pool.tile([P, S], BF, tag=f"attnT{c}",
                                     name=f"attnT{c}") for c in range(2)]

            # ---------------- emission pieces ----------------
            qk_rot = [0]

            def qk_proj(s4, m):
                rot = qk_rot[0]
                qk_rot[0] = (qk_rot[0] + 1) % ET
                ps = (psS if (s4 + m) % 2 else psD).tile(
                    [P, 512], F32, tag="sc", name=f"qk{s4}_{m}")
                for i in range(ET):
                    e = (rot + i) % ET
                    nc.tensor.matmul(
                        ps, lhsT=wqk[:, e, m * P:(m + 1) * P],
                        rhs=xT[:, e, s4 * 512:(s4 + 1) * 512],
                        start=(i == 0), stop=(i == ET - 1))
                nc.scalar.activation(qkT[m][s4], ps, IDENT,
                                     bias=bqk_sb[:, m:m + 1])

            def v_proj(st):
                psv = (psS if st % 2 else psD).tile(
                    [P, 512], F32, tag="sc", name=f"v{st}")
                for i in range(ET):
                    e = (qk_rot[0] + i) % ET
                    nc.tensor.matmul(
                        psv[:, 0:256], lhsT=xT[:, e, st * P:(st + 1) * P],
                        rhs=wv[:, e], start=(i == 0), stop=(i == ET - 1))
                nc.vector.tensor_copy(
                    vaug[:, st, :, 0:64],
                    psv[:, 0:256].rearrange("p (h d) -> p h d", h=HG))

            def scores_exp_half(h, q2, ks, ex, qh):
                """one [128,512] score half + its exp.
                even ks -> ACT true exp; odd ks -> DVE fast-exp."""
                pair, hp = h // 2, h % 2
                qm, km = pair, 2 + pair
                bp = hp * 64
                pool = psS if ks % 2 == 0 else psD
                scs = pool.tile([P, 512], F32, tag="sc",
                                name=f"sc{h}{q2}{ks}{qh}")
                nc.tensor.matmul(
                    scs,
                    lhsT=qkT[km][ks // 4][bp:bp + 64,
                                          (ks % 4) * P:(ks % 4 + 1) * P],
                    rhs=qkT[qm][q2 * 2 + qh][bp:bp + 64, :],
                    start=True, stop=True)
                qsl = slice(qh * 512, (qh + 1) * 512)
                if ks % 2 == 0 or ALLACT:
                    nc.scalar.activation(ex[:, qsl], scs, EXP,
                                         bias=ebias, scale=ESC)
                else:
                    nc.vector.tensor_scalar(
                        ex.bitcast(I16)[:, qsl], scs,
                        FEXP_K1, FEXP_K2, AluOpType.mult, AluOpType.add)

            deferred = []

            def attn_head(h, q2, exs, filler=None):
                for kp in range(ST // 2):
                    exa = ex_pool.tile([P, 1024], BF, tag="ex", name="exa")
                    exd = ex_pool.tile([P, 1024], BF, tag="ex", name="exd")
                    exs.extend([exa, exd])
                    for qh in range(2):
                        scores_exp_half(h, q2, 2 * kp, exa, qh)
                        scores_exp_half(h, q2, 2 * kp + 1, exd, qh)
                    if kp == 0 and deferred:
                        deferred.pop(0)()
                    if filler:
                        if kp == 7:
                            deferred.append(lambda f=filler: f(7))
                        else:
                            filler(kp)

            def pv_norm(h, q2, qt, exs):
                """PV + normalize for one query tile (128 q) of head h."""
                qs = slice((qt % 8) * P, (qt % 8 + 1) * P)
                pv = psV.tile([P, 512], F32, tag="pv", name=f"pv{h}{q2}{qt}")
                for ks, ex in enumerate(exs):
                    nc.tensor.matmul(
                        pv[:, 0:65], lhsT=ex[:, qs],
                        rhs=vaug[:, ks, h, :],
                        start=(ks == 0), stop=(ks == ST - 1))
                rb = rbp.tile([P, 1], F32, tag="rb", name="rb")
                nc.vector.reciprocal_approx_fast(rb, pv[:, 64:65])
                aq = attnq[qt % 8]
                nc.vector.tensor_scalar(
                    aq[:, h * 64:(h + 1) * 64], pv[:, 0:64],
                    rb, None, AluOpType.mult)

            def transpose_qt(qt):
                aq = attnq[qt % 8]
                tp = psD.tile([P, 512], BF, tag="sc", name=f"tp{qt}")
                for c in range(2):
                    nc.tensor.transpose(
                        tp[:, c * P:(c + 1) * P], aq[:, c * P:(c + 1) * P],
                        ident)
                for c in range(2):
                    nc.vector.tensor_copy(
                        attnT[c][:, qt * P:(qt + 1) * P],
                        tp[:, c * P:(c + 1) * P])

            def out_proj(qt, e2):
                po = (psS if e2 else psD).tile(
                    [P, 512], F32, tag="sc", name=f"po{qt}_{e2}")
                for i, c in enumerate((0, 1)):
                    nc.tensor.matmul(
                        po, lhsT=attnT[c][:, qt * P:(qt + 1) * P],
                        rhs=wout_sb[c][:, e2 * 512:(e2 + 1) * 512],
                        start=(i == 0), stop=(i == 1))
                ob = outsb.tile([P, 512], F32, tag="ob", name="ob")
                nc.scalar.activation(ob, po, COPY)
                (nc.sync if (qt + e2) % 2 else nc.gpsimd).dma_start(
                    out_d[qt * P:(qt + 1) * P, e2 * 512:(e2 + 1) * 512], ob)

            # ---------------- schedule ----------------
            attnq = [attnq_pool.tile([P, 256], BF, tag="aq", name=f"aq{i}")
                     for i in range(8)]

            for s4 in range(4):
                qk_proj(s4, 0)
            for s4 in range(4):
                qk_proj(s4, 2)

            ex_streams = {}
            done_pv = []
            tp_queue = []
            # heads-2/3 q/k projections deferred into stream (1,0)'s slots:
            # only stream (2,0)'s scores read them, and spreading them keeps
            # any single filler slot small enough not to starve the exp
            # engines. All v_proj stay in (0,0) so PV never reads an
            # unwritten vaug slot.
            qkm_queue = [(s4, m) for s4 in range(4) for m in (1, 3)]

            def drain_tp(n):
                # fine-grained: one piece (transpose or one out-proj half)
                # per call so PE filler load stays smooth across kp slots
                for _ in range(n):
                    if tp_queue:
                        tp_queue.pop(0)()

            for q2 in range(2):
                for h in range(4):
                    exs = []
                    ex_streams[(h, q2)] = exs

                    def filler(kp, h=h, q2=q2):
                        if q2 == 0 and h == 0:
                            v_proj(2 * kp)
                            v_proj(2 * kp + 1)
                            return
                        if qkm_queue:
                            s4m = qkm_queue.pop(0)
                            qk_proj(s4m[0], s4m[1])
                        drain_tp(2)
                        prev = (h - 1, q2) if h > 0 else (3, 0)
                        if prev in ex_streams and prev not in done_pv:
                            ph, pq2 = prev
                            qt = pq2 * 8 + kp
                            pv_norm(ph, pq2, qt, ex_streams[prev])
                            if ph == 3:
                                tp_queue.extend([
                                    lambda q=qt: transpose_qt(q),
                                    lambda q=qt: out_proj(q, 0),
                                    lambda q=qt: out_proj(q, 1)])
                            if kp == 7:
                                done_pv.append(prev)

                    attn_head(h, q2, exs, filler)

            # tail: per-qt chain of PV+norm -> transpose -> out-proj
            while deferred:
                deferred.pop(0)()
            drain_tp(len(tp_queue))
            for qt in range(8, 16):
                pv_norm(3, 1, qt, ex_streams[(3, 1)])
                transpose_qt(qt)
                out_proj(qt, 0)
                out_proj(qt, 1)

    nc.compile()
    return nc


def get_program():
    global _COMPILED
    if _COMPILED is None:
        _COMPILED = build_program()
    return _COMPILED


def make_in_maps(x, W_qkv, b_qkv, W_out, b_out):
    x = np.asarray(x, dtype=np.float32)
    W_qkv = np.asarray(W_qkv, dtype=np.float32)
    b_qkv = np.asarray(b_qkv, dtype=np.float32)
    W_out = np.asarray(W_out, dtype=np.float32)

    ident = np.eye(P, dtype=np.float32).astype(BF16)
    in_maps = []
    for c in range(N_CORES):
        b = c // 4
        g = c % 4
        heads = [4 * g + i for i in range(HG)]
        xT = np.ascontiguousarray(
            x[b].T.reshape(ET, P, S).transpose(1, 0, 2)).astype(BF16)

        qcols = np.empty((E, 512), np.float32)
        bqk = np.empty((P, 4), np.float32)
        vcols = np.empty((E, 256), np.float32)
        wout = np.empty((P, 2, E), np.float32)
        for m in range(4):
            for hl2 in range(2):
                head = heads[(m % 2) * 2 + hl2]
                base = head * 3 * D + (0 if m < 2 else D)
                qcols[:, m * P + hl2 * D: m * P + (hl2 + 1) * D] = \
                    W_qkv[:, base:base + D]
                bqk[hl2 * D:(hl2 + 1) * D, m] = b_qkv[base:base + D]
        for hl in range(HG):
            base = heads[hl] * 3 * D + 2 * D
            vcols[:, hl * D:(hl + 1) * D] = W_qkv[:, base:base + D]
        for cc in range(2):
            for f in range(P):
                wout[f, cc, :] = W_out[(heads[cc * 2 + f // D]) * D + f % D, :]

        wqk = np.ascontiguousarray(
            qcols.reshape(ET, P, 512).transpose(1, 0, 2)).astype(BF16)
        wv = np.ascontiguousarray(
            vcols.reshape(ET, P, 256).transpose(1, 0, 2)).astype(BF16)

        in_maps.append({
            "xT": xT,
            "wqk": wqk,
            "wv": wv,
            "wout": np.ascontiguousarray(wout).astype(BF16),
            "bqk": np.ascontiguousarray(bqk),
            "ident": ident,
        })
    return in_maps


def fold_bias(W_qkv, b_qkv, W_out, b_out):
    """b_out plus the V-bias contribution: softmax rows sum to 1, so each
    head's b_v passes straight through attention into the out-projection."""
    bv_all = b_qkv.reshape(H, 3 * D)[:, 2 * D:].reshape(E)
    return (b_out + bv_all @ W_out).astype(np.float32)


def gather_outputs(results, bias_const):
    out = np.zeros((B, S, E), np.float32)
    for c in range(N_CORES):
        out[c // 4] += results[c]["out"]
    return out + bias_const


def run(in_maps, trace=False, **kwargs):
    nc = get_program()
    return run_bass_kernel_spmd(nc, in_maps, list(range(N_CORES)),
                                trace=trace, **kwargs)


def kernel(x, W_qkv, b_qkv, W_out, b_out):
    W_qkv = np.asarray(W_qkv, np.float32)
    b_qkv = np.asarray(b_qkv, np.float32)
    W_out = np.asarray(W_out, np.float32)
    b_out = np.asarray(b_out, np.float32)
    in_maps = make_in_maps(x, W_qkv, b_qkv, W_out, b_out)
    res = run(in_maps)
    return gather_outputs(res.results, fold_bias(W_qkv, b_qkv, W_out, b_out))


# revision 8
# speedup vs baseline: 1.1726x; 1.0163x over previous
"""MultiHeadAttention forward on 8 Trainium2 NeuronCores — v3.

Problem: x[2,2048,1024] -> fused QKV proj -> 16-head attention -> out proj.
Sharding: (batch=2) x (head-groups=4) across 8 cores; core c handles batch
c//4 and heads 4*(c%4)..4*(c%4)+3. Host sums the 4 head-group partials.

Structure (optimized against the TimelineSim cost model, bf16 precision —
measured: fp8 projections/P/V all pass their elementwise error ~1:1 into the
attention output, so only bf16 survives the 2e-2 gate):
  - QKV projection bf16, feature-major q/k (scores operands), token-major V.
  - scores bf16 -> fp32 PSUM in [128,512] half-tiles; two decoupled 3-slot
    PSUM pools so the two exp engines pipeline independently.
  - exp split by key-tile parity: even ks -> ACT true exp; odd ks -> DVE
    fast-exp (one tensor_scalar mult+add -> int16 bits == bf16 exp values).
  - PV with queries on PSUM partitions: lhsT = exp-tile, rhs = V augmented
    with a ones column -> softmax denominators land as output column 64;
    normalize = reciprocal [128,1] + per-partition scalar multiply.
  - attn [q,f] -> PE transpose (identity) -> attnT [f,q] -> out-proj bf16.
  - b_v and b_out are folded on the host (softmax rows sum to 1, so both
    commute through attention into a constant added post-gather).
"""

import os
import numpy as np
import ml_dtypes

import concourse.bass as bass
import concourse.bacc as bacc
import concourse.tile as tile
from concourse import mybir
from concourse.alu_op_type import AluOpType
from concourse.bass_utils import run_bass_kernel_spmd

BF16 = ml_dtypes.bfloat16

B, S, E = 2, 2048, 1024
H, D = 16, 64
HG = 4               # heads per core
N_CORES = 8
P = 128
ST = S // P          # 16 token-tiles
ET = E // P          # 8 e-tiles

F32 = mybir.dt.float32
BF = mybir.dt.bfloat16
I16 = mybir.dt.int16
EXP = mybir.ActivationFunctionType.Exp
IDENT = mybir.ActivationFunctionType.Identity
COPY = mybir.ActivationFunctionType.Copy

ESC = 0.125          # exp(s_psum * ESC + EBIAS)
EBIAS = -2.0
LOG2E = 1.4426950408889634
# DVE fast-exp: u16 = round(s*K1 + K2); int16 bits == bf16(~exp(s/8 - 2))
FEXP_K1 = ESC * LOG2E * 128.0
FEXP_K2 = (127.0 + EBIAS * LOG2E) * 128.0 - 5.7

ALLACT = bool(os.environ.get("ALLACT"))

_COMPILED = None


def build_program():
    nc = bacc.Bacc("TRN2", target_bir_lowering=False, debug=False)

    xT_d = nc.dram_tensor("xT", [P, ET, S], BF, kind="ExternalInput").ap()
    wqk_d = nc.dram_tensor("wqk", [P, ET, 512], BF, kind="ExternalInput").ap()
    wv_d = nc.dram_tensor("wv", [P, ET, 256], BF, kind="ExternalInput").ap()
    wout_d = nc.dram_tensor("wout", [P, 2, E], BF, kind="ExternalInput").ap()
    bqk_d = nc.dram_tensor("bqk", [P, 4], F32, kind="ExternalInput").ap()
    ident_d = nc.dram_tensor("ident", [P, P], BF, kind="ExternalInput").ap()
    out_d = nc.dram_tensor("out", [S, E], F32, kind="ExternalOutput").ap()

    with tile.TileContext(nc) as tc:
        with (
            tc.tile_pool(name="consts", bufs=1) as consts,
            tc.tile_pool(name="qkt", bufs=1) as qkt_pool,
            tc.tile_pool(name="exp", bufs=34) as ex_pool,
            tc.tile_pool(name="attnq", bufs=9) as attnq_pool,
            tc.tile_pool(name="attnt", bufs=1) as attnt_pool,
            tc.tile_pool(name="rbp", bufs=14) as rbp,
            tc.tile_pool(name="outsb", bufs=12) as outsb,
            tc.tile_pool(name="psS", bufs=3, space="PSUM") as psS,
            tc.tile_pool(name="psD", bufs=3, space="PSUM") as psD,
            tc.tile_pool(name="psV", bufs=2, space="PSUM") as psV,
        ):
            # ---------------- constants / inputs ----------------
            wqk = consts.tile([P, ET, 512], BF, tag="wqk")
            nc.sync.dma_start(wqk[:, 0:4], wqk_d[:, 0:4])
            bqk_sb = consts.tile([P, 4], F32, tag="bqk")
            nc.gpsimd.dma_start(bqk_sb, bqk_d)
            xT = consts.tile([P, ET, S], BF, tag="xT", name="xT")
            for e in range(ET):
                (nc.sync if e % 2 else nc.gpsimd).dma_start(
                    xT[:, e:e + 1], xT_d[:, e:e + 1])
            nc.sync.dma_start(wqk[:, 4:8], wqk_d[:, 4:8])
            wv = consts.tile([P, ET, 256], BF, tag="wv")
            nc.gpsimd.dma_start(wv, wv_d)
            wout_all = consts.tile([P, 2, E], BF, tag="wout")
            nc.sync.dma_start(wout_all, wout_d)
            wout_sb = [wout_all[:, c, :] for c in range(2)]
            ident = consts.tile([P, P], BF, tag="ident")
            nc.gpsimd.dma_start(ident, ident_d)
            ebias = consts.tile([P, 1], F32, tag="ebias")
            nc.vector.memset(ebias, EBIAS)
            warm = consts.tile([P, 1], BF, tag="warm")
            nc.scalar.activation(warm, ebias, EXP)

            # persistent activations
            # qkT m-tiles: 0=q(h0,h1) 1=q(h2,h3) 2=k(h0,h1) 3=k(h2,h3);
            # partitions 0:64 = even head, 64:128 = odd head; bf16.
            qkT = [[qkt_pool.tile([P, 512], BF, tag=f"qkT{m}_{s4}",
                                  name=f"qkT{m}_{s4}")
                    for s4 in range(4)] for m in range(4)]
            # V augmented: [128 tok, st, head, 65]; col 64 = 1.0 (denom row)
            vaug = consts.tile([P, ST, HG, 65], BF, tag="vaug")
            nc.vector.memset(vaug[:, :, :, 64:65], 1.0)
            # attnT [f, q] for out-proj: c=0 heads {0,1}, c=1 heads {2,3}
            attnT = [attnt_pool.tile([P, S], BF, tag=f"attnT{c}",
                                     name=f"attnT{c}") for c in range(2)]

            # ---------------- emission pieces ----------------
            qk_rot = [0]

            def qk_proj(s4, m):
                rot = qk_rot[0]
                qk_rot[0] = (qk_rot[0] + 1) % ET
                ps = (psS if (s4 + m) % 2 else psD).tile(
                    [P, 512], F32, tag="sc", name=f"qk{s4}_{m}")
                for i in range(ET):
                    e = (rot + i) % ET
                    nc.tensor.matmul(
                        ps, lhsT=wqk[:, e, m * P:(m + 1) * P],
                        rhs=xT[:, e, s4 * 512:(s4 + 1) * 512],
                        start=(i == 0), stop=(i == ET - 1))
                nc.scalar.activation(qkT[m][s4], ps, IDENT,
                                     bias=bqk_sb[:, m:m + 1])

            def v_proj(st):
                psv = (psS if st % 2 else psD).tile(
                    [P, 512], F32, tag="sc", name=f"v{st}")
                for i in range(ET):
                    e = (qk_rot[0] + i) % ET
                    nc.tensor.matmul(
                        psv[:, 0:256], lhsT=xT[:, e, st * P:(st + 1) * P],
                        rhs=wv[:, e], start=(i == 0), stop=(i == ET - 1))
                nc.vector.tensor_copy(
                    vaug[:, st, :, 0:64],
                    psv[:, 0:256].rearrange("p (h d) -> p h d", h=HG))

            def scores_exp_half(h, q2, ks, ex, qh):
                """one [128,512] score half + its exp.
                even ks -> ACT true exp; odd ks -> DVE fast-exp."""
                pair, hp = h // 2, h % 2
                qm, km = pair, 2 + pair
                bp = hp * 64
                pool = psS if ks % 2 == 0 else psD
                scs = pool.tile([P, 512], F32, tag="sc",
                                name=f"sc{h}{q2}{ks}{qh}")
                nc.tensor.matmul(
                    scs,
                    lhsT=qkT[km][ks // 4][bp:bp + 64,
                                          (ks % 4) * P:(ks % 4 + 1) * P],
                    rhs=qkT[qm][q2 * 2 + qh][bp:bp + 64, :],
                    start=True, stop=True)
                qsl = slice(qh * 512, (qh + 1) * 512)
                if ks % 2 == 0 or ALLACT:
                    nc.scalar.activation(ex[:, qsl], scs, EXP,
                                         bias=ebias, scale=ESC)
                else:
                    nc.vector.tensor_scalar(
                        ex.bitcast(I16)[:, qsl], scs,
                        FEXP_K1, FEXP_K2, AluOpType.mult, AluOpType.add)

            deferred = []

            def attn_head(h, q2, exs, filler=None):
                for kp in range(ST // 2):
                    exa = ex_pool.tile([P, 1024], BF, tag="ex", name="exa")
                    exd = ex_pool.tile([P, 1024], BF, tag="ex", name="exd")
                    exs.extend([exa, exd])
                    for qh in range(2):
                        scores_exp_half(h, q2, 2 * kp, exa, qh)
                        scores_exp_half(h, q2, 2 * kp + 1, exd, qh)
                    if kp == 0 and deferred:
                        deferred.pop(0)()
                    if filler:
                        if kp == 7:
                            deferred.append(lambda f=filler: f(7))
                        else:
                            filler(kp)

            def pv_norm(h, q2, qt, exs):
                """PV + normalize for one query tile (128 q) of head h."""
                qs = slice((qt % 8) * P, (qt % 8 + 1) * P)
                pv = psV.tile([P, 512], F32, tag="pv", name=f"pv{h}{q2}{qt}")
                for ks, ex in enumerate(exs):
                    nc.tensor.matmul(
                        pv[:, 0:65], lhsT=ex[:, qs],
                        rhs=vaug[:, ks, h, :],
                        start=(ks == 0), stop=(ks == ST - 1))
                rb = rbp.tile([P, 1], F32, tag="rb", name="rb")
                nc.vector.reciprocal_approx_fast(rb, pv[:, 64:65])
                aq = attnq[qt % 8]
                nc.vector.tensor_scalar(
                    aq[:, h * 64:(h + 1) * 64], pv[:, 0:64],
                    rb, None, AluOpType.mult)

            def transpose_qt(qt):
                aq = attnq[qt % 8]
                tp = psD.tile([P, 512], BF, tag="sc", name=f"tp{qt}")
                for c in range(2):
                    nc.tensor.transpose(
                        tp[:, c * P:(c + 1) * P], aq[:, c * P:(c + 1) * P],
                        ident)
                for c in range(2):
                    nc.vector.tensor_copy(
                        attnT[c][:, qt * P:(qt + 1) * P],
                        tp[:, c * P:(c + 1) * P])

            def out_proj(qt, e2):
                po = (psS if e2 else psD).tile(
                    [P, 512], F32, tag="sc", name=f"po{qt}_{e2}")
                for i, c in enumerate((0, 1)):
                    nc.tensor.matmul(
                        po, lhsT=attnT[c][:, qt * P:(qt + 1) * P],
                        rhs=wout_sb[c][:, e2 * 512:(e2 + 1) * 512],
                        start=(i == 0), stop=(i == 1))
                ob = outsb.tile([P, 512], F32, tag="ob", name="ob")
                nc.scalar.activation(ob, po, COPY)
                (nc.sync if (qt + e2) % 2 else nc.gpsimd).dma_start(
                    out_d[qt * P:(qt + 1) * P, e2 * 512:(e2 + 1) * 512], ob)

            # ---------------- schedule ----------------
            attnq = [attnq_pool.tile([P, 256], BF, tag="aq", name=f"aq{i}")
                     for i in range(8)]

            for s4 in range(4):
                qk_proj(s4, 0)
                qk_proj(s4, 2)

            ex_streams = {}
            done_pv = []
            tp_queue = []
            # heads-2/3 q/k projections deferred into stream (1,0)'s slots:
            # only stream (2,0)'s scores read them, and spreading them keeps
            # any single filler slot small enough not to starve the exp
            # engines. All v_proj stay in (0,0) so PV never reads an
            # unwritten vaug slot.
            qkm_queue = [(s4, m) for s4 in range(4) for m in (1, 3)]

            def drain_tp(n):
                # fine-grained: one piece (transpose or one out-proj half)
                # per call so PE filler load stays smooth across kp slots
                for _ in range(n):
                    if tp_queue:
                        tp_queue.pop(0)()

            for q2 in range(2):
                for h in range(4):
                    exs = []
                    ex_streams[(h, q2)] = exs

                    def filler(kp, h=h, q2=q2):
                        if q2 == 0 and h == 0:
                            v_proj(2 * kp)
                            v_proj(2 * kp + 1)
                            return
                        if qkm_queue:
                            s4m = qkm_queue.pop(0)
                            qk_proj(s4m[0], s4m[1])
                        drain_tp(2)
                        prev = (h - 1, q2) if h > 0 else (3, 0)
                        if prev in ex_streams and prev not in done_pv:
                            ph, pq2 = prev
                            qt = pq2 * 8 + kp
                            pv_norm(ph, pq2, qt, ex_streams[prev])
                            if ph == 3:
                                tp_queue.extend([
                                    lambda q=qt: transpose_qt(q),
                                    lambda q=qt: out_proj(q, 0),
                                    lambda q=qt: out_proj(q, 1)])
                            if kp == 7:
                                done_pv.append(prev)

                    attn_head(h, q2, exs, filler)

            # tail: per-qt chain of PV+norm -> transpose -> out-proj
            while deferred:
                deferred.pop(0)()
            drain_tp(len(tp_queue))
            for qt in range(8, 16):
                pv_norm(3, 1, qt, ex_streams[(3, 1)])
                transpose_qt(qt)
                out_proj(qt, 0)
                out_proj(qt, 1)

    nc.compile()
    return nc


def get_program():
    global _COMPILED
    if _COMPILED is None:
        _COMPILED = build_program()
    return _COMPILED


def make_in_maps(x, W_qkv, b_qkv, W_out, b_out):
    x = np.asarray(x, dtype=np.float32)
    W_qkv = np.asarray(W_qkv, dtype=np.float32)
    b_qkv = np.asarray(b_qkv, dtype=np.float32)
    W_out = np.asarray(W_out, dtype=np.float32)

    ident = np.eye(P, dtype=np.float32).astype(BF16)
    in_maps = []
    for c in range(N_CORES):
        b = c // 4
        g = c % 4
        heads = [4 * g + i for i in range(HG)]
        xT = np.ascontiguousarray(
            x[b].T.reshape(ET, P, S).transpose(1, 0, 2)).astype(BF16)

        qcols = np.empty((E, 512), np.float32)
        bqk = np.empty((P, 4), np.float32)
        vcols = np.empty((E, 256), np.float32)
        wout = np.empty((P, 2, E), np.float32)
        for m in range(4):
            for hl2 in range(2):
                head = heads[(m % 2) * 2 + hl2]
                base = head * 3 * D + (0 if m < 2 else D)
                qcols[:, m * P + hl2 * D: m * P + (hl2 + 1) * D] = \
                    W_qkv[:, base:base + D]
                bqk[hl2 * D:(hl2 + 1) * D, m] = b_qkv[base:base + D]
        for hl in range(HG):
            base = heads[hl] * 3 * D + 2 * D
            vcols[:, hl * D:(hl + 1) * D] = W_qkv[:, base:base + D]
        for cc in range(2):
            for f in range(P):
                wout[f, cc, :] = W_out[(heads[cc * 2 + f // D]) * D + f % D, :]

        wqk = np.ascontiguousarray(
            qcols.reshape(ET, P, 512).transpose(1, 0, 2)).astype(BF16)
        wv = np.ascontiguousarray(
            vcols.reshape(ET, P, 256).transpose(1, 0, 2)).astype(BF16)

        in_maps.append({
            "xT": xT,
            "wqk": wqk,
            "wv": wv,
            "wout": np.ascontiguousarray(wout).astype(BF16),
            "bqk": np.ascontiguousarray(bqk),
            "ident": ident,
        })
    return in_maps


def fold_bias(W_qkv, b_qkv, W_out, b_out):
    """b_out plus the V-bias contribution: softmax rows sum to 1, so each
    head's b_v passes straight through attention into the out-projection."""
    bv_all = b_qkv.reshape(H, 3 * D)[:, 2 * D:].reshape(E)
    return (b_out + bv_all @ W_out).astype(np.float32)


def gather_outputs(results, bias_const):
    out = np.zeros((B, S, E), np.float32)
    for c in range(N_CORES):
        out[c // 4] += results[c]["out"]
    return out + bias_const


def run(in_maps, trace=False, **kwargs):
    nc = get_program()
    return run_bass_kernel_spmd(nc, in_maps, list(range(N_CORES)),
                                trace=trace, **kwargs)


def kernel(x, W_qkv, b_qkv, W_out, b_out):
    W_qkv = np.asarray(W_qkv, np.float32)
    b_qkv = np.asarray(b_qkv, np.float32)
    W_out = np.asarray(W_out, np.float32)
    b_out = np.asarray(b_out, np.float32)
    in_maps = make_in_maps(x, W_qkv, b_qkv, W_out, b_out)
    res = run(in_maps)
    return gather_outputs(res.results, fold_bias(W_qkv, b_qkv, W_out, b_out))


# revision 9
# speedup vs baseline: 1.1755x; 1.0025x over previous
"""MultiHeadAttention forward on 8 Trainium2 NeuronCores — v3.

Problem: x[2,2048,1024] -> fused QKV proj -> 16-head attention -> out proj.
Sharding: (batch=2) x (head-groups=4) across 8 cores; core c handles batch
c//4 and heads 4*(c%4)..4*(c%4)+3. Host sums the 4 head-group partials.

Structure (optimized against the TimelineSim cost model, bf16 precision —
measured: fp8 projections/P/V all pass their elementwise error ~1:1 into the
attention output, so only bf16 survives the 2e-2 gate):
  - QKV projection bf16, feature-major q/k (scores operands), token-major V.
  - scores bf16 -> fp32 PSUM in [128,512] half-tiles; two decoupled 3-slot
    PSUM pools so the two exp engines pipeline independently.
  - exp split by key-tile parity: even ks -> ACT true exp; odd ks -> DVE
    fast-exp (one tensor_scalar mult+add -> int16 bits == bf16 exp values).
  - PV with queries on PSUM partitions: lhsT = exp-tile, rhs = V augmented
    with a ones column -> softmax denominators land as output column 64;
    normalize = reciprocal [128,1] + per-partition scalar multiply.
  - attn [q,f] -> PE transpose (identity) -> attnT [f,q] -> out-proj bf16.
  - b_v and b_out are folded on the host (softmax rows sum to 1, so both
    commute through attention into a constant added post-gather).
"""

import os
import numpy as np
import ml_dtypes

import concourse.bass as bass
import concourse.bacc as bacc
import concourse.tile as tile
from concourse import mybir
from concourse.alu_op_type import AluOpType
from concourse.bass_utils import run_bass_kernel_spmd

BF16 = ml_dtypes.bfloat16

B, S, E = 2, 2048, 1024
H, D = 16, 64
HG = 4               # heads per core
N_CORES = 8
P = 128
ST = S // P          # 16 token-tiles
ET = E // P          # 8 e-tiles

F32 = mybir.dt.float32
BF = mybir.dt.bfloat16
I16 = mybir.dt.int16
EXP = mybir.ActivationFunctionType.Exp
IDENT = mybir.ActivationFunctionType.Identity
COPY = mybir.ActivationFunctionType.Copy

ESC = 0.125          # exp(s_psum * ESC + EBIAS)
EBIAS = -2.0
LOG2E = 1.4426950408889634
# DVE fast-exp: u16 = round(s*K1 + K2); int16 bits == bf16(~exp(s/8 - 2))
FEXP_K1 = ESC * LOG2E * 128.0
FEXP_K2 = (127.0 + EBIAS * LOG2E) * 128.0 - 5.7

ALLACT = bool(os.environ.get("ALLACT"))

_COMPILED = None


def build_program():
    nc = bacc.Bacc("TRN2", target_bir_lowering=False, debug=False)

    xT_d = nc.dram_tensor("xT", [P, ET, S], BF, kind="ExternalInput").ap()
    wqk_d = nc.dram_tensor("wqk", [P, ET, 512], BF, kind="ExternalInput").ap()
    wv_d = nc.dram_tensor("wv", [P, ET, 256], BF, kind="ExternalInput").ap()
    wout_d = nc.dram_tensor("wout", [P, 2, E], BF, kind="ExternalInput").ap()
    bqk_d = nc.dram_tensor("bqk", [P, 4], F32, kind="ExternalInput").ap()
    ident_d = nc.dram_tensor("ident", [P, P], BF, kind="ExternalInput").ap()
    out_d = nc.dram_tensor("out", [S, E], F32, kind="ExternalOutput").ap()

    with tile.TileContext(nc) as tc:
        with (
            tc.tile_pool(name="consts", bufs=1) as consts,
            tc.tile_pool(name="qkt", bufs=1) as qkt_pool,
            tc.tile_pool(name="exp", bufs=34) as ex_pool,
            tc.tile_pool(name="attnq", bufs=9) as attnq_pool,
            tc.tile_pool(name="attnt", bufs=1) as attnt_pool,
            tc.tile_pool(name="rbp", bufs=14) as rbp,
            tc.tile_pool(name="outsb", bufs=12) as outsb,
            tc.tile_pool(name="psS", bufs=3, space="PSUM") as psS,
            tc.tile_pool(name="psD", bufs=3, space="PSUM") as psD,
            tc.tile_pool(name="psV", bufs=2, space="PSUM") as psV,
        ):
            # ---------------- constants / inputs ----------------
            wqk = consts.tile([P, ET, 512], BF, tag="wqk")
            nc.sync.dma_start(wqk[:, 0:4], wqk_d[:, 0:4])
            bqk_sb = consts.tile([P, 4], F32, tag="bqk")
            nc.gpsimd.dma_start(bqk_sb, bqk_d)
            xT = consts.tile([P, ET, S], BF, tag="xT", name="xT")
            for e in range(ET):
                (nc.sync if e % 2 else nc.gpsimd).dma_start(
                    xT[:, e:e + 1], xT_d[:, e:e + 1])
            nc.sync.dma_start(wqk[:, 4:8], wqk_d[:, 4:8])
            wv = consts.tile([P, ET, 256], BF, tag="wv")
            nc.gpsimd.dma_start(wv, wv_d)
            wout_all = consts.tile([P, 2, E], BF, tag="wout")
            nc.sync.dma_start(wout_all, wout_d)
            wout_sb = [wout_all[:, c, :] for c in range(2)]
            ident = consts.tile([P, P], BF, tag="ident")
            nc.gpsimd.dma_start(ident, ident_d)
            ebias = consts.tile([P, 1], F32, tag="ebias")
            nc.vector.memset(ebias, EBIAS)
            warm = consts.tile([P, 1], BF, tag="warm")
            nc.scalar.activation(warm, ebias, EXP)

            # persistent activations
            # qkT m-tiles: 0=q(h0,h1) 1=q(h2,h3) 2=k(h0,h1) 3=k(h2,h3);
            # partitions 0:64 = even head, 64:128 = odd head; bf16.
            qkT = [[qkt_pool.tile([P, 512], BF, tag=f"qkT{m}_{s4}",
                                  name=f"qkT{m}_{s4}")
                    for s4 in range(4)] for m in range(4)]
            # V augmented: [128 tok, st, head, 65]; col 64 = 1.0 (denom row)
            vaug = consts.tile([P, ST, HG, 65], BF, tag="vaug")
            nc.vector.memset(vaug[:, :, :, 64:65], 1.0)
            # attnT [f, q] for out-proj: c=0 heads {0,1}, c=1 heads {2,3}
            attnT = [attnt_pool.tile([P, S], BF, tag=f"attnT{c}",
                                     name=f"attnT{c}") for c in range(2)]

            # ---------------- emission pieces ----------------
            qk_rot = [0]

            def qk_proj(s4, m):
                rot = qk_rot[0]
                qk_rot[0] = (qk_rot[0] + 1) % ET
                ps = (psS if (s4 + m) % 2 else psD).tile(
                    [P, 512], F32, tag="sc", name=f"qk{s4}_{m}")
                for i in range(ET):
                    e = (rot + i) % ET
                    nc.tensor.matmul(
                        ps, lhsT=wqk[:, e, m * P:(m + 1) * P],
                        rhs=xT[:, e, s4 * 512:(s4 + 1) * 512],
                        start=(i == 0), stop=(i == ET - 1))
                nc.scalar.activation(qkT[m][s4], ps, IDENT,
                                     bias=bqk_sb[:, m:m + 1])

            def v_proj(st):
                psv = (psS if st % 2 else psD).tile(
                    [P, 512], F32, tag="sc", name=f"v{st}")
                for i in range(ET):
                    e = (qk_rot[0] + i) % ET
                    nc.tensor.matmul(
                        psv[:, 0:256], lhsT=xT[:, e, st * P:(st + 1) * P],
                        rhs=wv[:, e], start=(i == 0), stop=(i == ET - 1))
                nc.vector.tensor_copy(
                    vaug[:, st, :, 0:64],
                    psv[:, 0:256].rearrange("p (h d) -> p h d", h=HG))

            def scores_exp_half(h, q2, ks, ex, qh):
                """one [128,512] score half + its exp.
                even ks -> ACT true exp; odd ks -> DVE fast-exp."""
                pair, hp = h // 2, h % 2
                qm, km = pair, 2 + pair
                bp = hp * 64
                pool = psS if (ks % 2 == 0 or ks == 15) else psD
                scs = pool.tile([P, 512], F32, tag="sc",
                                name=f"sc{h}{q2}{ks}{qh}")
                nc.tensor.matmul(
                    scs,
                    lhsT=qkT[km][ks // 4][bp:bp + 64,
                                          (ks % 4) * P:(ks % 4 + 1) * P],
                    rhs=qkT[qm][q2 * 2 + qh][bp:bp + 64, :],
                    start=True, stop=True)
                qsl = slice(qh * 512, (qh + 1) * 512)
                if (ks % 2 == 0 or ks == 15) or ALLACT:
                    nc.scalar.activation(ex[:, qsl], scs, EXP,
                                         bias=ebias, scale=ESC)
                else:
                    nc.vector.tensor_scalar(
                        ex.bitcast(I16)[:, qsl], scs,
                        FEXP_K1, FEXP_K2, AluOpType.mult, AluOpType.add)

            deferred = []

            def attn_head(h, q2, exs, filler=None):
                for kp in range(ST // 2):
                    exa = ex_pool.tile([P, 1024], BF, tag="ex", name="exa")
                    exd = ex_pool.tile([P, 1024], BF, tag="ex", name="exd")
                    exs.extend([exa, exd])
                    for qh in range(2):
                        scores_exp_half(h, q2, 2 * kp, exa, qh)
                        scores_exp_half(h, q2, 2 * kp + 1, exd, qh)
                    if kp == 0 and deferred:
                        deferred.pop(0)()
                    if filler:
                        if kp == 7:
                            deferred.append(lambda f=filler: f(7))
                        else:
                            filler(kp)

            def pv_norm(h, q2, qt, exs):
                """PV + normalize for one query tile (128 q) of head h."""
                qs = slice((qt % 8) * P, (qt % 8 + 1) * P)
                pv = psV.tile([P, 512], F32, tag="pv", name=f"pv{h}{q2}{qt}")
                for ks, ex in enumerate(exs):
                    nc.tensor.matmul(
                        pv[:, 0:65], lhsT=ex[:, qs],
                        rhs=vaug[:, ks, h, :],
                        start=(ks == 0), stop=(ks == ST - 1))
                rb = rbp.tile([P, 1], F32, tag="rb", name="rb")
                nc.vector.reciprocal_approx_fast(rb, pv[:, 64:65])
                aq = attnq[qt % 8]
                nc.vector.tensor_scalar(
                    aq[:, h * 64:(h + 1) * 64], pv[:, 0:64],
                    rb, None, AluOpType.mult)

            def transpose_qt(qt):
                aq = attnq[qt % 8]
                tp = psD.tile([P, 512], BF, tag="sc", name=f"tp{qt}")
                for c in range(2):
                    nc.tensor.transpose(
                        tp[:, c * P:(c + 1) * P], aq[:, c * P:(c + 1) * P],
                        ident)
                for c in range(2):
                    nc.vector.tensor_copy(
                        attnT[c][:, qt * P:(qt + 1) * P],
                        tp[:, c * P:(c + 1) * P])

            def out_proj(qt, e2):
                po = (psS if e2 else psD).tile(
                    [P, 512], F32, tag="sc", name=f"po{qt}_{e2}")
                for i, c in enumerate((0, 1)):
                    nc.tensor.matmul(
                        po, lhsT=attnT[c][:, qt * P:(qt + 1) * P],
                        rhs=wout_sb[c][:, e2 * 512:(e2 + 1) * 512],
                        start=(i == 0), stop=(i == 1))
                ob = outsb.tile([P, 512], F32, tag="ob", name="ob")
                nc.scalar.activation(ob, po, COPY)
                (nc.sync if (qt + e2) % 2 else nc.gpsimd).dma_start(
                    out_d[qt * P:(qt + 1) * P, e2 * 512:(e2 + 1) * 512], ob)

            # ---------------- schedule ----------------
            attnq = [attnq_pool.tile([P, 256], BF, tag="aq", name=f"aq{i}")
                     for i in range(8)]

            for s4 in range(4):
                qk_proj(s4, 0)
                qk_proj(s4, 2)

            ex_streams = {}
            done_pv = []
            tp_queue = []
            # heads-2/3 q/k projections deferred into stream (1,0)'s slots:
            # only stream (2,0)'s scores read them, and spreading them keeps
            # any single filler slot small enough not to starve the exp
            # engines. All v_proj stay in (0,0) so PV never reads an
            # unwritten vaug slot.
            qkm_queue = [(s4, m) for s4 in range(4) for m in (1, 3)]

            def drain_tp(n):
                # fine-grained: one piece (transpose or one out-proj half)
                # per call so PE filler load stays smooth across kp slots
                for _ in range(n):
                    if tp_queue:
                        tp_queue.pop(0)()

            for q2 in range(2):
                for h in range(4):
                    exs = []
                    ex_streams[(h, q2)] = exs

                    def filler(kp, h=h, q2=q2):
                        if q2 == 0 and h == 0:
                            v_proj(2 * kp)
                            v_proj(2 * kp + 1)
                            return
                        if qkm_queue:
                            s4m = qkm_queue.pop(0)
                            qk_proj(s4m[0], s4m[1])
                        drain_tp(2)
                        prev = (h - 1, q2) if h > 0 else (3, 0)
                        if prev in ex_streams and prev not in done_pv:
                            ph, pq2 = prev
                            qt = pq2 * 8 + kp
                            pv_norm(ph, pq2, qt, ex_streams[prev])
                            if ph == 3:
                                tp_queue.extend([
                                    lambda q=qt: transpose_qt(q),
                                    lambda q=qt: out_proj(q, 0),
                                    lambda q=qt: out_proj(q, 1)])
                            if kp == 7:
                                done_pv.append(prev)

                    attn_head(h, q2, exs, filler)

            # tail: per-qt chain of PV+norm -> transpose -> out-proj
            while deferred:
                deferred.pop(0)()
            drain_tp(len(tp_queue))
            for qt in range(8, 16):
                pv_norm(3, 1, qt, ex_streams[(3, 1)])
                transpose_qt(qt)
                out_proj(qt, 0)
                out_proj(qt, 1)

    nc.compile()
    return nc


def get_program():
    global _COMPILED
    if _COMPILED is None:
        _COMPILED = build_program()
    return _COMPILED


def make_in_maps(x, W_qkv, b_qkv, W_out, b_out):
    x = np.asarray(x, dtype=np.float32)
    W_qkv = np.asarray(W_qkv, dtype=np.float32)
    b_qkv = np.asarray(b_qkv, dtype=np.float32)
    W_out = np.asarray(W_out, dtype=np.float32)

    ident = np.eye(P, dtype=np.float32).astype(BF16)
    in_maps = []
    for c in range(N_CORES):
        b = c // 4
        g = c % 4
        heads = [4 * g + i for i in range(HG)]
        xT = np.ascontiguousarray(
            x[b].T.reshape(ET, P, S).transpose(1, 0, 2)).astype(BF16)

        qcols = np.empty((E, 512), np.float32)
        bqk = np.empty((P, 4), np.float32)
        vcols = np.empty((E, 256), np.float32)
        wout = np.empty((P, 2, E), np.float32)
        for m in range(4):
            for hl2 in range(2):
                head = heads[(m % 2) * 2 + hl2]
                base = head * 3 * D + (0 if m < 2 else D)
                qcols[:, m * P + hl2 * D: m * P + (hl2 + 1) * D] = \
                    W_qkv[:, base:base + D]
                bqk[hl2 * D:(hl2 + 1) * D, m] = b_qkv[base:base + D]
        for hl in range(HG):
            base = heads[hl] * 3 * D + 2 * D
            vcols[:, hl * D:(hl + 1) * D] = W_qkv[:, base:base + D]
        for cc in range(2):
            for f in range(P):
                wout[f, cc, :] = W_out[(heads[cc * 2 + f // D]) * D + f % D, :]

        wqk = np.ascontiguousarray(
            qcols.reshape(ET, P, 512).transpose(1, 0, 2)).astype(BF16)
        wv = np.ascontiguousarray(
            vcols.reshape(ET, P, 256).transpose(1, 0, 2)).astype(BF16)

        in_maps.append({
            "xT": xT,
            "wqk": wqk,
            "wv": wv,
            "wout": np.ascontiguousarray(wout).astype(BF16),
            "bqk": np.ascontiguousarray(bqk),
            "ident": ident,
        })
    return in_maps


def fold_bias(W_qkv, b_qkv, W_out, b_out):
    """b_out plus the V-bias contribution: softmax rows sum to 1, so each
    head's b_v passes straight through attention into the out-projection."""
    bv_all = b_qkv.reshape(H, 3 * D)[:, 2 * D:].reshape(E)
    return (b_out + bv_all @ W_out).astype(np.float32)


def gather_outputs(results, bias_const):
    out = np.zeros((B, S, E), np.float32)
    for c in range(N_CORES):
        out[c // 4] += results[c]["out"]
    return out + bias_const


def run(in_maps, trace=False, **kwargs):
    nc = get_program()
    return run_bass_kernel_spmd(nc, in_maps, list(range(N_CORES)),
                                trace=trace, **kwargs)


def kernel(x, W_qkv, b_qkv, W_out, b_out):
    W_qkv = np.asarray(W_qkv, np.float32)
    b_qkv = np.asarray(b_qkv, np.float32)
    W_out = np.asarray(W_out, np.float32)
    b_out = np.asarray(b_out, np.float32)
    in_maps = make_in_maps(x, W_qkv, b_qkv, W_out, b_out)
    res = run(in_maps)
    return gather_outputs(res.results, fold_bias(W_qkv, b_qkv, W_out, b_out))


# revision 10
# speedup vs baseline: 1.2051x; 1.0252x over previous
"""MultiHeadAttention forward on 8 Trainium2 NeuronCores — v3.

Problem: x[2,2048,1024] -> fused QKV proj -> 16-head attention -> out proj.
Sharding: (batch=2) x (head-groups=4) across 8 cores; core c handles batch
c//4 and heads 4*(c%4)..4*(c%4)+3. Host sums the 4 head-group partials.

Structure (optimized against the TimelineSim cost model, bf16 precision —
measured: fp8 projections/P/V all pass their elementwise error ~1:1 into the
attention output, so only bf16 survives the 2e-2 gate):
  - QKV projection bf16, feature-major q/k (scores operands), token-major V.
  - scores bf16 -> fp32 PSUM in [128,512] half-tiles; two decoupled 3-slot
    PSUM pools so the two exp engines pipeline independently.
  - exp split by key-tile parity: even ks -> ACT true exp; odd ks -> DVE
    fast-exp (one tensor_scalar mult+add -> int16 bits == bf16 exp values).
  - PV with queries on PSUM partitions: lhsT = exp-tile, rhs = V augmented
    with a ones column -> softmax denominators land as output column 64;
    normalize = reciprocal [128,1] + per-partition scalar multiply.
  - attn [q,f] -> PE transpose (identity) -> attnT [f,q] -> out-proj bf16.
  - b_v and b_out are folded on the host (softmax rows sum to 1, so both
    commute through attention into a constant added post-gather).
"""

import os
import numpy as np
import ml_dtypes

import concourse.bass as bass
import concourse.bacc as bacc
import concourse.tile as tile
from concourse import mybir
from concourse.alu_op_type import AluOpType
from concourse.bass_utils import run_bass_kernel_spmd

BF16 = ml_dtypes.bfloat16

B, S, E = 2, 2048, 1024
H, D = 16, 64
HG = 4               # heads per core
N_CORES = 8
P = 128
ST = S // P          # 16 token-tiles
ET = E // P          # 8 e-tiles

F32 = mybir.dt.float32
BF = mybir.dt.bfloat16
I16 = mybir.dt.int16
EXP = mybir.ActivationFunctionType.Exp
IDENT = mybir.ActivationFunctionType.Identity
COPY = mybir.ActivationFunctionType.Copy

ESC = 0.125          # exp(s_psum * ESC + EBIAS)
EBIAS = -2.0
LOG2E = 1.4426950408889634
# DVE fast-exp: u16 = round(s*K1 + K2); int16 bits == bf16(~exp(s/8 - 2))
FEXP_K1 = ESC * LOG2E * 128.0
FEXP_K2 = (127.0 + EBIAS * LOG2E) * 128.0 - 5.7

ALLACT = bool(os.environ.get("ALLACT"))

_COMPILED = None


def build_program():
    nc = bacc.Bacc("TRN2", target_bir_lowering=False, debug=False)

    xT_d = nc.dram_tensor("xT", [P, ET, S], BF, kind="ExternalInput").ap()
    wqk_d = nc.dram_tensor("wqk", [P, 2, ET, 256], BF,
                           kind="ExternalInput").ap()
    wv_d = nc.dram_tensor("wv", [P, ET, 256], BF, kind="ExternalInput").ap()
    wout_d = nc.dram_tensor("wout", [P, 2, E], BF, kind="ExternalInput").ap()
    bqk_d = nc.dram_tensor("bqk", [P, 4], F32, kind="ExternalInput").ap()
    ident_d = nc.dram_tensor("ident", [P, P], BF, kind="ExternalInput").ap()
    out_d = nc.dram_tensor("out", [S, E], F32, kind="ExternalOutput").ap()

    with tile.TileContext(nc) as tc:
        with (
            tc.tile_pool(name="consts", bufs=1) as consts,
            tc.tile_pool(name="qkt", bufs=1) as qkt_pool,
            tc.tile_pool(name="exp", bufs=34) as ex_pool,
            tc.tile_pool(name="attnq", bufs=9) as attnq_pool,
            tc.tile_pool(name="attnt", bufs=1) as attnt_pool,
            tc.tile_pool(name="rbp", bufs=14) as rbp,
            tc.tile_pool(name="outsb", bufs=12) as outsb,
            tc.tile_pool(name="psS", bufs=3, space="PSUM") as psS,
            tc.tile_pool(name="psD", bufs=3, space="PSUM") as psD,
            tc.tile_pool(name="psV", bufs=2, space="PSUM") as psV,
        ):
            # ---------------- constants / inputs ----------------
            wqk = consts.tile([P, 2, ET, 256], BF, tag="wqk")
            nc.sync.dma_start(wqk[:, 0], wqk_d[:, 0])
            bqk_sb = consts.tile([P, 4], F32, tag="bqk")
            nc.gpsimd.dma_start(bqk_sb, bqk_d)
            xT = consts.tile([P, ET, S], BF, tag="xT", name="xT")
            for e in range(ET):
                (nc.sync if e % 2 else nc.gpsimd).dma_start(
                    xT[:, e:e + 1], xT_d[:, e:e + 1])
            nc.sync.dma_start(wqk[:, 1], wqk_d[:, 1])
            wv = consts.tile([P, ET, 256], BF, tag="wv")
            nc.gpsimd.dma_start(wv, wv_d)
            wout_all = consts.tile([P, 2, E], BF, tag="wout")
            nc.sync.dma_start(wout_all, wout_d)
            wout_sb = [wout_all[:, c, :] for c in range(2)]
            ident = consts.tile([P, P], BF, tag="ident")
            nc.gpsimd.dma_start(ident, ident_d)
            ebias = consts.tile([P, 1], F32, tag="ebias")
            nc.vector.memset(ebias, EBIAS)
            warm = consts.tile([P, 1], BF, tag="warm")
            nc.scalar.activation(warm, ebias, EXP)

            # persistent activations
            # qkT m-tiles: 0=q(h0,h1) 1=q(h2,h3) 2=k(h0,h1) 3=k(h2,h3);
            # partitions 0:64 = even head, 64:128 = odd head; bf16.
            qkT = [[qkt_pool.tile([P, 512], BF, tag=f"qkT{m}_{s4}",
                                  name=f"qkT{m}_{s4}")
                    for s4 in range(4)] for m in range(4)]
            # V augmented: [128 tok, st, head, 65]; col 64 = 1.0 (denom row)
            vaug = consts.tile([P, ST, HG, 65], BF, tag="vaug")
            nc.vector.memset(vaug[:, :, :, 64:65], 1.0)
            # attnT [f, q] for out-proj: c=0 heads {0,1}, c=1 heads {2,3}
            attnT = [attnt_pool.tile([P, S], BF, tag=f"attnT{c}",
                                     name=f"attnT{c}") for c in range(2)]

            # ---------------- emission pieces ----------------
            qk_rot = [0]

            def qk_proj(s4, m):
                rot = qk_rot[0]
                qk_rot[0] = (qk_rot[0] + 1) % ET
                ps = (psS if (s4 + m) % 2 else psD).tile(
                    [P, 512], F32, tag="sc", name=f"qk{s4}_{m}")
                for i in range(ET):
                    e = (rot + i) % ET
                    nc.tensor.matmul(
                        ps, lhsT=wqk[:, m % 2, e,
                                     (m // 2) * P:(m // 2 + 1) * P],
                        rhs=xT[:, e, s4 * 512:(s4 + 1) * 512],
                        start=(i == 0), stop=(i == ET - 1))
                nc.scalar.activation(qkT[m][s4], ps, IDENT,
                                     bias=bqk_sb[:, m:m + 1])

            def v_proj(st):
                psv = (psS if st % 2 else psD).tile(
                    [P, 512], F32, tag="sc", name=f"v{st}")
                for i in range(ET):
                    e = (qk_rot[0] + i) % ET
                    nc.tensor.matmul(
                        psv[:, 0:256], lhsT=xT[:, e, st * P:(st + 1) * P],
                        rhs=wv[:, e], start=(i == 0), stop=(i == ET - 1))
                nc.vector.tensor_copy(
                    vaug[:, st, :, 0:64],
                    psv[:, 0:256].rearrange("p (h d) -> p h d", h=HG))

            def scores_exp_half(h, q2, ks, ex, qh):
                """one [128,512] score half + its exp.
                even ks -> ACT true exp; odd ks -> DVE fast-exp."""
                pair, hp = h // 2, h % 2
                qm, km = pair, 2 + pair
                bp = hp * 64
                pool = psS if (ks % 2 == 0 or ks == 15) else psD
                scs = pool.tile([P, 512], F32, tag="sc",
                                name=f"sc{h}{q2}{ks}{qh}")
                nc.tensor.matmul(
                    scs,
                    lhsT=qkT[km][ks // 4][bp:bp + 64,
                                          (ks % 4) * P:(ks % 4 + 1) * P],
                    rhs=qkT[qm][q2 * 2 + qh][bp:bp + 64, :],
                    start=True, stop=True)
                qsl = slice(qh * 512, (qh + 1) * 512)
                if (ks % 2 == 0 or ks == 15) or ALLACT:
                    nc.scalar.activation(ex[:, qsl], scs, EXP,
                                         bias=ebias, scale=ESC)
                else:
                    nc.vector.tensor_scalar(
                        ex.bitcast(I16)[:, qsl], scs,
                        FEXP_K1, FEXP_K2, AluOpType.mult, AluOpType.add)

            deferred = []

            def attn_head(h, q2, exs, filler=None):
                for kp in range(ST // 2):
                    exa = ex_pool.tile([P, 1024], BF, tag="ex", name="exa")
                    exd = ex_pool.tile([P, 1024], BF, tag="ex", name="exd")
                    exs.extend([exa, exd])
                    for qh in range(2):
                        scores_exp_half(h, q2, 2 * kp, exa, qh)
                        scores_exp_half(h, q2, 2 * kp + 1, exd, qh)
                    if kp == 0 and deferred:
                        deferred.pop(0)()
                    if filler:
                        if kp == 7:
                            deferred.append(lambda f=filler: f(7))
                        else:
                            filler(kp)

            def pv_norm(h, q2, qt, exs):
                """PV + normalize for one query tile (128 q) of head h."""
                qs = slice((qt % 8) * P, (qt % 8 + 1) * P)
                pv = psV.tile([P, 512], F32, tag="pv", name=f"pv{h}{q2}{qt}")
                for ks, ex in enumerate(exs):
                    nc.tensor.matmul(
                        pv[:, 0:65], lhsT=ex[:, qs],
                        rhs=vaug[:, ks, h, :],
                        start=(ks == 0), stop=(ks == ST - 1))
                rb = rbp.tile([P, 1], F32, tag="rb", name="rb")
                nc.vector.reciprocal_approx_fast(rb, pv[:, 64:65])
                aq = attnq[qt % 8]
                nc.vector.tensor_scalar(
                    aq[:, h * 64:(h + 1) * 64], pv[:, 0:64],
                    rb, None, AluOpType.mult)

            def transpose_qt(qt):
                aq = attnq[qt % 8]
                tp = psD.tile([P, 512], BF, tag="sc", name=f"tp{qt}")
                for c in range(2):
                    nc.tensor.transpose(
                        tp[:, c * P:(c + 1) * P], aq[:, c * P:(c + 1) * P],
                        ident)
                for c in range(2):
                    nc.vector.tensor_copy(
                        attnT[c][:, qt * P:(qt + 1) * P],
                        tp[:, c * P:(c + 1) * P])

            def out_proj(qt, e2):
                po = (psS if e2 else psD).tile(
                    [P, 512], F32, tag="sc", name=f"po{qt}_{e2}")
                for i, c in enumerate((0, 1)):
                    nc.tensor.matmul(
                        po, lhsT=attnT[c][:, qt * P:(qt + 1) * P],
                        rhs=wout_sb[c][:, e2 * 512:(e2 + 1) * 512],
                        start=(i == 0), stop=(i == 1))
                ob = outsb.tile([P, 512], F32, tag="ob", name="ob")
                nc.scalar.activation(ob, po, COPY)
                (nc.sync if (qt + e2) % 2 else nc.gpsimd).dma_start(
                    out_d[qt * P:(qt + 1) * P, e2 * 512:(e2 + 1) * 512], ob)

            # ---------------- schedule ----------------
            attnq = [attnq_pool.tile([P, 256], BF, tag="aq", name=f"aq{i}")
                     for i in range(8)]

            for s4 in range(4):
                qk_proj(s4, 0)
                qk_proj(s4, 2)

            ex_streams = {}
            done_pv = []
            tp_queue = []
            # heads-2/3 q/k projections deferred into stream (1,0)'s slots:
            # only stream (2,0)'s scores read them, and spreading them keeps
            # any single filler slot small enough not to starve the exp
            # engines. All v_proj stay in (0,0) so PV never reads an
            # unwritten vaug slot.
            qkm_queue = [(s4, m) for s4 in range(4) for m in (1, 3)]

            def drain_tp(n):
                # fine-grained: one piece (transpose or one out-proj half)
                # per call so PE filler load stays smooth across kp slots
                for _ in range(n):
                    if tp_queue:
                        tp_queue.pop(0)()

            for q2 in range(2):
                for h in range(4):
                    exs = []
                    ex_streams[(h, q2)] = exs

                    def filler(kp, h=h, q2=q2):
                        if q2 == 0 and h == 0:
                            v_proj(2 * kp)
                            v_proj(2 * kp + 1)
                            return
                        if qkm_queue:
                            s4m = qkm_queue.pop(0)
                            qk_proj(s4m[0], s4m[1])
                        drain_tp(2)
                        prev = (h - 1, q2) if h > 0 else (3, 0)
                        if prev in ex_streams and prev not in done_pv:
                            ph, pq2 = prev
                            qt = pq2 * 8 + kp
                            pv_norm(ph, pq2, qt, ex_streams[prev])
                            if ph == 3:
                                tp_queue.extend([
                                    lambda q=qt: transpose_qt(q),
                                    lambda q=qt: out_proj(q, 0),
                                    lambda q=qt: out_proj(q, 1)])
                            if kp == 7:
                                done_pv.append(prev)

                    attn_head(h, q2, exs, filler)

            # tail: per-qt chain of PV+norm -> transpose -> out-proj
            while deferred:
                deferred.pop(0)()
            drain_tp(len(tp_queue))
            for qt in range(8, 16):
                pv_norm(3, 1, qt, ex_streams[(3, 1)])
                transpose_qt(qt)
                out_proj(qt, 0)
                out_proj(qt, 1)

    nc.compile()
    return nc


def get_program():
    global _COMPILED
    if _COMPILED is None:
        _COMPILED = build_program()
    return _COMPILED


def make_in_maps(x, W_qkv, b_qkv, W_out, b_out):
    x = np.asarray(x, dtype=np.float32)
    W_qkv = np.asarray(W_qkv, dtype=np.float32)
    b_qkv = np.asarray(b_qkv, dtype=np.float32)
    W_out = np.asarray(W_out, dtype=np.float32)

    ident = np.eye(P, dtype=np.float32).astype(BF16)
    in_maps = []
    for c in range(N_CORES):
        b = c // 4
        g = c % 4
        heads = [4 * g + i for i in range(HG)]
        xT = np.ascontiguousarray(
            x[b].T.reshape(ET, P, S).transpose(1, 0, 2)).astype(BF16)

        qcols = np.empty((E, 2, 256), np.float32)
        bqk = np.empty((P, 4), np.float32)
        vcols = np.empty((E, 256), np.float32)
        wout = np.empty((P, 2, E), np.float32)
        for m in range(4):
            for hl2 in range(2):
                head = heads[(m % 2) * 2 + hl2]
                base = head * 3 * D + (0 if m < 2 else D)
                qcols[:, m % 2, (m // 2) * P + hl2 * D:
                      (m // 2) * P + (hl2 + 1) * D] = \
                    W_qkv[:, base:base + D]
                bqk[hl2 * D:(hl2 + 1) * D, m] = b_qkv[base:base + D]
        for hl in range(HG):
            base = heads[hl] * 3 * D + 2 * D
            vcols[:, hl * D:(hl + 1) * D] = W_qkv[:, base:base + D]
        for cc in range(2):
            for f in range(P):
                wout[f, cc, :] = W_out[(heads[cc * 2 + f // D]) * D + f % D, :]

        wqk = np.ascontiguousarray(
            qcols.reshape(ET, P, 2, 256).transpose(1, 2, 0, 3)).astype(BF16)
        wv = np.ascontiguousarray(
            vcols.reshape(ET, P, 256).transpose(1, 0, 2)).astype(BF16)

        in_maps.append({
            "xT": xT,
            "wqk": wqk,
            "wv": wv,
            "wout": np.ascontiguousarray(wout).astype(BF16),
            "bqk": np.ascontiguousarray(bqk),
            "ident": ident,
        })
    return in_maps


def fold_bias(W_qkv, b_qkv, W_out, b_out):
    """b_out plus the V-bias contribution: softmax rows sum to 1, so each
    head's b_v passes straight through attention into the out-projection."""
    bv_all = b_qkv.reshape(H, 3 * D)[:, 2 * D:].reshape(E)
    return (b_out + bv_all @ W_out).astype(np.float32)


def gather_outputs(results, bias_const):
    out = np.zeros((B, S, E), np.float32)
    for c in range(N_CORES):
        out[c // 4] += results[c]["out"]
    return out + bias_const


def run(in_maps, trace=False, **kwargs):
    nc = get_program()
    return run_bass_kernel_spmd(nc, in_maps, list(range(N_CORES)),
                                trace=trace, **kwargs)


def kernel(x, W_qkv, b_qkv, W_out, b_out):
    W_qkv = np.asarray(W_qkv, np.float32)
    b_qkv = np.asarray(b_qkv, np.float32)
    W_out = np.asarray(W_out, np.float32)
    b_out = np.asarray(b_out, np.float32)
    in_maps = make_in_maps(x, W_qkv, b_qkv, W_out, b_out)
    res = run(in_maps)
    return gather_outputs(res.results, fold_bias(W_qkv, b_qkv, W_out, b_out))


# revision 11
# speedup vs baseline: 1.2146x; 1.0079x over previous
"""MultiHeadAttention forward on 8 Trainium2 NeuronCores — v3.

Problem: x[2,2048,1024] -> fused QKV proj -> 16-head attention -> out proj.
Sharding: (batch=2) x (head-groups=4) across 8 cores; core c handles batch
c//4 and heads 4*(c%4)..4*(c%4)+3. Host sums the 4 head-group partials.

Structure (optimized against the TimelineSim cost model, bf16 precision —
measured: fp8 projections/P/V all pass their elementwise error ~1:1 into the
attention output, so only bf16 survives the 2e-2 gate):
  - QKV projection bf16, feature-major q/k (scores operands), token-major V.
  - scores bf16 -> fp32 PSUM in [128,512] half-tiles; two decoupled 3-slot
    PSUM pools so the two exp engines pipeline independently.
  - exp split by key-tile parity: even ks -> ACT true exp; odd ks -> DVE
    fast-exp (one tensor_scalar mult+add -> int16 bits == bf16 exp values).
  - PV with queries on PSUM partitions: lhsT = exp-tile, rhs = V augmented
    with a ones column -> softmax denominators land as output column 64;
    normalize = reciprocal [128,1] + per-partition scalar multiply.
  - attn [q,f] -> PE transpose (identity) -> attnT [f,q] -> out-proj bf16.
  - b_v and b_out are folded on the host (softmax rows sum to 1, so both
    commute through attention into a constant added post-gather).
"""

import os
import numpy as np
import ml_dtypes

import concourse.bass as bass
import concourse.bacc as bacc
import concourse.tile as tile
from concourse import mybir
from concourse.alu_op_type import AluOpType
from concourse.bass_utils import run_bass_kernel_spmd

BF16 = ml_dtypes.bfloat16

B, S, E = 2, 2048, 1024
H, D = 16, 64
HG = 4               # heads per core
N_CORES = 8
P = 128
ST = S // P          # 16 token-tiles
ET = E // P          # 8 e-tiles

F32 = mybir.dt.float32
BF = mybir.dt.bfloat16
I16 = mybir.dt.int16
EXP = mybir.ActivationFunctionType.Exp
IDENT = mybir.ActivationFunctionType.Identity
COPY = mybir.ActivationFunctionType.Copy

ESC = 0.125          # exp(s_psum * ESC + EBIAS)
EBIAS = -2.0
LOG2E = 1.4426950408889634
# DVE fast-exp: u16 = round(s*K1 + K2); int16 bits == bf16(~exp(s/8 - 2))
FEXP_K1 = ESC * LOG2E * 128.0
FEXP_K2 = (127.0 + EBIAS * LOG2E) * 128.0 - 5.7

ALLACT = bool(os.environ.get("ALLACT"))

_COMPILED = None


def build_program():
    nc = bacc.Bacc("TRN2", target_bir_lowering=False, debug=False)

    xT_d = nc.dram_tensor("xT", [P, ET, S], BF, kind="ExternalInput").ap()
    wqk_d = nc.dram_tensor("wqk", [P, 2, ET, 256], BF,
                           kind="ExternalInput").ap()
    wv_d = nc.dram_tensor("wv", [P, ET, 256], BF, kind="ExternalInput").ap()
    wout_d = nc.dram_tensor("wout", [P, 2, E], BF, kind="ExternalInput").ap()
    bqk_d = nc.dram_tensor("bqk", [P, 4], F32, kind="ExternalInput").ap()
    ident_d = nc.dram_tensor("ident", [P, P], BF, kind="ExternalInput").ap()
    out_d = nc.dram_tensor("out", [S, E], F32, kind="ExternalOutput").ap()

    with tile.TileContext(nc) as tc:
        with (
            tc.tile_pool(name="consts", bufs=1) as consts,
            tc.tile_pool(name="qkt", bufs=1) as qkt_pool,
            tc.tile_pool(name="exp", bufs=34) as ex_pool,
            tc.tile_pool(name="attnq", bufs=9) as attnq_pool,
            tc.tile_pool(name="attnt", bufs=1) as attnt_pool,
            tc.tile_pool(name="rbp", bufs=14) as rbp,
            tc.tile_pool(name="outsb", bufs=12) as outsb,
            tc.tile_pool(name="psS", bufs=3, space="PSUM") as psS,
            tc.tile_pool(name="psD", bufs=3, space="PSUM") as psD,
            tc.tile_pool(name="psV", bufs=2, space="PSUM") as psV,
        ):
            # ---------------- constants / inputs ----------------
            wqk = consts.tile([P, 2, ET, 256], BF, tag="wqk")
            nc.sync.dma_start(wqk[:, 0], wqk_d[:, 0])
            bqk_sb = consts.tile([P, 4], F32, tag="bqk")
            nc.gpsimd.dma_start(bqk_sb, bqk_d)
            xT = consts.tile([P, ET, S], BF, tag="xT", name="xT")
            # token-halves: the s4 0/1 projection groups (which gate the
            # first scores) read only columns 0:1024 of every e-chunk, so
            # all first halves load before any second half
            for e in range(ET):
                (nc.sync if e % 2 else nc.gpsimd).dma_start(
                    xT[:, e:e + 1, 0:1024], xT_d[:, e:e + 1, 0:1024])
            for e in range(ET):
                (nc.sync if e % 2 else nc.gpsimd).dma_start(
                    xT[:, e:e + 1, 1024:2048], xT_d[:, e:e + 1, 1024:2048])
            nc.sync.dma_start(wqk[:, 1], wqk_d[:, 1])
            wv = consts.tile([P, ET, 256], BF, tag="wv")
            nc.gpsimd.dma_start(wv, wv_d)
            wout_all = consts.tile([P, 2, E], BF, tag="wout")
            nc.sync.dma_start(wout_all, wout_d)
            wout_sb = [wout_all[:, c, :] for c in range(2)]
            ident = consts.tile([P, P], BF, tag="ident")
            nc.gpsimd.dma_start(ident, ident_d)
            ebias = consts.tile([P, 1], F32, tag="ebias")
            nc.vector.memset(ebias, EBIAS)
            warm = consts.tile([P, 1], BF, tag="warm")
            nc.scalar.activation(warm, ebias, EXP)

            # persistent activations
            # qkT m-tiles: 0=q(h0,h1) 1=q(h2,h3) 2=k(h0,h1) 3=k(h2,h3);
            # partitions 0:64 = even head, 64:128 = odd head; bf16.
            qkT = [[qkt_pool.tile([P, 512], BF, tag=f"qkT{m}_{s4}",
                                  name=f"qkT{m}_{s4}")
                    for s4 in range(4)] for m in range(4)]
            # V augmented: [128 tok, st, head, 65]; col 64 = 1.0 (denom row)
            vaug = consts.tile([P, ST, HG, 65], BF, tag="vaug")
            nc.vector.memset(vaug[:, :, :, 64:65], 1.0)
            # attnT [f, q] for out-proj: c=0 heads {0,1}, c=1 heads {2,3}
            attnT = [attnt_pool.tile([P, S], BF, tag=f"attnT{c}",
                                     name=f"attnT{c}") for c in range(2)]

            # ---------------- emission pieces ----------------
            qk_rot = [0]

            def qk_proj(s4, m):
                rot = qk_rot[0]
                qk_rot[0] = (qk_rot[0] + 1) % ET
                ps = (psS if (s4 + m) % 2 else psD).tile(
                    [P, 512], F32, tag="sc", name=f"qk{s4}_{m}")
                for i in range(ET):
                    e = (rot + i) % ET
                    nc.tensor.matmul(
                        ps, lhsT=wqk[:, m % 2, e,
                                     (m // 2) * P:(m // 2 + 1) * P],
                        rhs=xT[:, e, s4 * 512:(s4 + 1) * 512],
                        start=(i == 0), stop=(i == ET - 1))
                nc.scalar.activation(qkT[m][s4], ps, IDENT,
                                     bias=bqk_sb[:, m:m + 1])

            def v_proj(st):
                psv = (psS if st % 2 else psD).tile(
                    [P, 512], F32, tag="sc", name=f"v{st}")
                for i in range(ET):
                    e = (qk_rot[0] + i) % ET
                    nc.tensor.matmul(
                        psv[:, 0:256], lhsT=xT[:, e, st * P:(st + 1) * P],
                        rhs=wv[:, e], start=(i == 0), stop=(i == ET - 1))
                nc.vector.tensor_copy(
                    vaug[:, st, :, 0:64],
                    psv[:, 0:256].rearrange("p (h d) -> p h d", h=HG))

            def scores_exp_half(h, q2, ks, ex, qh):
                """one [128,512] score half + its exp.
                even ks -> ACT true exp; odd ks -> DVE fast-exp."""
                pair, hp = h // 2, h % 2
                qm, km = pair, 2 + pair
                bp = hp * 64
                pool = psS if (ks % 2 == 0 or ks == 15) else psD
                scs = pool.tile([P, 512], F32, tag="sc",
                                name=f"sc{h}{q2}{ks}{qh}")
                nc.tensor.matmul(
                    scs,
                    lhsT=qkT[km][ks // 4][bp:bp + 64,
                                          (ks % 4) * P:(ks % 4 + 1) * P],
                    rhs=qkT[qm][q2 * 2 + qh][bp:bp + 64, :],
                    start=True, stop=True)
                qsl = slice(qh * 512, (qh + 1) * 512)
                if (ks % 2 == 0 or ks == 15) or ALLACT:
                    nc.scalar.activation(ex[:, qsl], scs, EXP,
                                         bias=ebias, scale=ESC)
                else:
                    nc.vector.tensor_scalar(
                        ex.bitcast(I16)[:, qsl], scs,
                        FEXP_K1, FEXP_K2, AluOpType.mult, AluOpType.add)

            deferred = []

            def attn_head(h, q2, exs, filler=None):
                for kp in range(ST // 2):
                    exa = ex_pool.tile([P, 1024], BF, tag="ex", name="exa")
                    exd = ex_pool.tile([P, 1024], BF, tag="ex", name="exd")
                    exs.extend([exa, exd])
                    for qh in range(2):
                        scores_exp_half(h, q2, 2 * kp, exa, qh)
                        scores_exp_half(h, q2, 2 * kp + 1, exd, qh)
                    if kp == 0 and deferred:
                        deferred.pop(0)()
                    if filler:
                        if kp == 7:
                            deferred.append(lambda f=filler: f(7))
                        else:
                            filler(kp)

            def pv_norm(h, q2, qt, exs):
                """PV + normalize for one query tile (128 q) of head h."""
                qs = slice((qt % 8) * P, (qt % 8 + 1) * P)
                pv = psV.tile([P, 512], F32, tag="pv", name=f"pv{h}{q2}{qt}")
                for ks, ex in enumerate(exs):
                    nc.tensor.matmul(
                        pv[:, 0:65], lhsT=ex[:, qs],
                        rhs=vaug[:, ks, h, :],
                        start=(ks == 0), stop=(ks == ST - 1))
                rb = rbp.tile([P, 1], F32, tag="rb", name="rb")
                nc.vector.reciprocal_approx_fast(rb, pv[:, 64:65])
                aq = attnq[qt % 8]
                nc.vector.tensor_scalar(
                    aq[:, h * 64:(h + 1) * 64], pv[:, 0:64],
                    rb, None, AluOpType.mult)

            def transpose_qt(qt):
                aq = attnq[qt % 8]
                tp = psD.tile([P, 512], BF, tag="sc", name=f"tp{qt}")
                for c in range(2):
                    nc.tensor.transpose(
                        tp[:, c * P:(c + 1) * P], aq[:, c * P:(c + 1) * P],
                        ident)
                for c in range(2):
                    nc.vector.tensor_copy(
                        attnT[c][:, qt * P:(qt + 1) * P],
                        tp[:, c * P:(c + 1) * P])

            def out_proj(qt, e2):
                po = (psS if e2 else psD).tile(
                    [P, 512], F32, tag="sc", name=f"po{qt}_{e2}")
                for i, c in enumerate((0, 1)):
                    nc.tensor.matmul(
                        po, lhsT=attnT[c][:, qt * P:(qt + 1) * P],
                        rhs=wout_sb[c][:, e2 * 512:(e2 + 1) * 512],
                        start=(i == 0), stop=(i == 1))
                ob = outsb.tile([P, 512], F32, tag="ob", name="ob")
                nc.scalar.activation(ob, po, COPY)
                (nc.sync if (qt + e2) % 2 else nc.gpsimd).dma_start(
                    out_d[qt * P:(qt + 1) * P, e2 * 512:(e2 + 1) * 512], ob)

            # ---------------- schedule ----------------
            attnq = [attnq_pool.tile([P, 256], BF, tag="aq", name=f"aq{i}")
                     for i in range(8)]

            for s4 in range(4):
                if s4 < 2:
                    qk_proj(s4, 0)
                qk_proj(s4, 2)

            ex_streams = {}
            done_pv = []
            tp_queue = []
            # heads-2/3 q/k projections deferred into stream (1,0)'s slots:
            # only stream (2,0)'s scores read them, and spreading them keeps
            # any single filler slot small enough not to starve the exp
            # engines. All v_proj stay in (0,0) so PV never reads an
            # unwritten vaug slot.
            qkm_queue = [(2, 0), (3, 0)] + \
                [(s4, m) for s4 in range(4) for m in (1, 3)]

            def drain_tp(n):
                # fine-grained: one piece (transpose or one out-proj half)
                # per call so PE filler load stays smooth across kp slots
                for _ in range(n):
                    if tp_queue:
                        tp_queue.pop(0)()

            for q2 in range(2):
                for h in range(4):
                    exs = []
                    ex_streams[(h, q2)] = exs

                    def filler(kp, h=h, q2=q2):
                        if q2 == 0 and h == 0:
                            v_proj(2 * kp)
                            v_proj(2 * kp + 1)
                            return
                        if qkm_queue:
                            s4m = qkm_queue.pop(0)
                            qk_proj(s4m[0], s4m[1])
                        drain_tp(2)
                        prev = (h - 1, q2) if h > 0 else (3, 0)
                        if prev in ex_streams and prev not in done_pv:
                            ph, pq2 = prev
                            qt = pq2 * 8 + kp
                            pv_norm(ph, pq2, qt, ex_streams[prev])
                            if ph == 3:
                                tp_queue.extend([
                                    lambda q=qt: transpose_qt(q),
                                    lambda q=qt: out_proj(q, 0),
                                    lambda q=qt: out_proj(q, 1)])
                            if kp == 7:
                                done_pv.append(prev)

                    attn_head(h, q2, exs, filler)

            # tail: per-qt chain of PV+norm -> transpose -> out-proj
            while deferred:
                deferred.pop(0)()
            drain_tp(len(tp_queue))
            for qt in range(8, 16):
                pv_norm(3, 1, qt, ex_streams[(3, 1)])
                transpose_qt(qt)
                out_proj(qt, 0)
                out_proj(qt, 1)

    nc.compile()
    return nc


def get_program():
    global _COMPILED
    if _COMPILED is None:
        _COMPILED = build_program()
    return _COMPILED


def make_in_maps(x, W_qkv, b_qkv, W_out, b_out):
    x = np.asarray(x, dtype=np.float32)
    W_qkv = np.asarray(W_qkv, dtype=np.float32)
    b_qkv = np.asarray(b_qkv, dtype=np.float32)
    W_out = np.asarray(W_out, dtype=np.float32)

    ident = np.eye(P, dtype=np.float32).astype(BF16)
    in_maps = []
    for c in range(N_CORES):
        b = c // 4
        g = c % 4
        heads = [4 * g + i for i in range(HG)]
        xT = np.ascontiguousarray(
            x[b].T.reshape(ET, P, S).transpose(1, 0, 2)).astype(BF16)

        qcols = np.empty((E, 2, 256), np.float32)
        bqk = np.empty((P, 4), np.float32)
        vcols = np.empty((E, 256), np.float32)
        wout = np.empty((P, 2, E), np.float32)
        for m in range(4):
            for hl2 in range(2):
                head = heads[(m % 2) * 2 + hl2]
                base = head * 3 * D + (0 if m < 2 else D)
                qcols[:, m % 2, (m // 2) * P + hl2 * D:
                      (m // 2) * P + (hl2 + 1) * D] = \
                    W_qkv[:, base:base + D]
                bqk[hl2 * D:(hl2 + 1) * D, m] = b_qkv[base:base + D]
        for hl in range(HG):
            base = heads[hl] * 3 * D + 2 * D
            vcols[:, hl * D:(hl + 1) * D] = W_qkv[:, base:base + D]
        for cc in range(2):
            for f in range(P):
                wout[f, cc, :] = W_out[(heads[cc * 2 + f // D]) * D + f % D, :]

        wqk = np.ascontiguousarray(
            qcols.reshape(ET, P, 2, 256).transpose(1, 2, 0, 3)).astype(BF16)
        wv = np.ascontiguousarray(
            vcols.reshape(ET, P, 256).transpose(1, 0, 2)).astype(BF16)

        in_maps.append({
            "xT": xT,
            "wqk": wqk,
            "wv": wv,
            "wout": np.ascontiguousarray(wout).astype(BF16),
            "bqk": np.ascontiguousarray(bqk),
            "ident": ident,
        })
    return in_maps


def fold_bias(W_qkv, b_qkv, W_out, b_out):
    """b_out plus the V-bias contribution: softmax rows sum to 1, so each
    head's b_v passes straight through attention into the out-projection."""
    bv_all = b_qkv.reshape(H, 3 * D)[:, 2 * D:].reshape(E)
    return (b_out + bv_all @ W_out).astype(np.float32)


def gather_outputs(results, bias_const):
    out = np.zeros((B, S, E), np.float32)
    for c in range(N_CORES):
        out[c // 4] += results[c]["out"]
    return out + bias_const


def run(in_maps, trace=False, **kwargs):
    nc = get_program()
    return run_bass_kernel_spmd(nc, in_maps, list(range(N_CORES)),
                                trace=trace, **kwargs)


def kernel(x, W_qkv, b_qkv, W_out, b_out):
    W_qkv = np.asarray(W_qkv, np.float32)
    b_qkv = np.asarray(b_qkv, np.float32)
    W_out = np.asarray(W_out, np.float32)
    b_out = np.asarray(b_out, np.float32)
    in_maps = make_in_maps(x, W_qkv, b_qkv, W_out, b_out)
    res = run(in_maps)
    return gather_outputs(res.results, fold_bias(W_qkv, b_qkv, W_out, b_out))
